# revision 1
# baseline (speedup 1.0000x reference)
"""MiniMax Lightning Attention on 8 Trainium2 NeuronCores.

Sharding: sequence-parallel. Core c handles batch c//4, token chunk
(c%4)*1024..+1024 (4 blocks of 256). The per-block decay-state recurrence
crosses chunk boundaries; each core computes its local per-chunk decay-
weighted KV summary E, an AllGather shares the 8 summaries, and each core
reconstructs its chunk-start state as a decay-weighted sum.

All matmuls run as fp32r (full-rate fp32 on the PE at N>=256).
"""

import numpy as np

from contextlib import ExitStack

import concourse.bacc as bacc
import concourse.mybir as mybir
import concourse.tile as tile
from concourse.bass_utils import run_bass_kernel_spmd
from concourse.masks import make_identity

AF = mybir.ActivationFunctionType
ALU = mybir.AluOpType
F32 = mybir.dt.float32
F32R = mybir.dt.float32r

H = 32
D = 64
BS = 256
HID = 2048
B = 2
S = 4096
NC = 8
T = S // 4            # tokens per core (1024)
NCH = T // 128        # 8 token chunks of 128
NBLK = T // BS        # 4 blocks per core
LAYER_IDX = 0
NUM_LAYERS = 32
EPS = 1e-5


def _decay():
    base = 1.0 / 2.0 ** (8.0 / H)
    rate = base ** (np.arange(H, dtype=np.float64) + 1.0)
    factor = 1.0 - LAYER_IDX / (NUM_LAYERS - 1 + 1e-5) + 1e-5
    slope = rate * factor                                  # (H,)
    r = np.arange(BS, dtype=np.float64) + 1.0
    qd = np.exp(-slope[:, None] * r[None, :])              # (H, BS) query decay
    kd = np.exp(-slope[:, None] * (BS - r[None, :]))       # (H, BS) key decay
    ij = r[:, None] - r[None, :]                           # i - j
    dd = np.where(
        ij[None] >= 0, np.exp(-slope[:, None, None] * ij[None]), 0.0
    )                                                      # (H, BS_i, BS_j)
    bd = np.exp(-slope * BS)                               # (H,) block decay
    return slope, qd, kd, dd, bd


def _build_nc():
    nc = bacc.Bacc(num_devices=NC)
    hsT = nc.declare_dram_parameter("hsT", [HID, T], F32R, isOutput=False)
    wqkT = nc.declare_dram_parameter("wqkT", [HID, 2 * H * D], F32R, isOutput=False)
    wvT = nc.declare_dram_parameter("wvT", [HID, H * D], F32R, isOutput=False)
    gwT = nc.declare_dram_parameter("gwT", [HID, HID], F32R, isOutput=False)
    owT = nc.declare_dram_parameter("owT", [H * D, HID], F32R, isOutput=False)
    ddm = nc.declare_dram_parameter("ddm", [H, 2, 128, BS], F32, isOutput=False)
    qdm = nc.declare_dram_parameter("qdm", [H, D, BS], F32, isOutput=False)
    kdm = nc.declare_dram_parameter("kdm", [128, 2 * H], F32, isOutput=False)
    nw = nc.declare_dram_parameter("nw", [128, 16], F32, isOutput=False)
    swm = nc.declare_dram_parameter("swm", [D, H * NC], F32, isOutput=False)
    out = nc.declare_dram_parameter("out", [T, HID], F32, isOutput=True)

    qk_spill = nc.dram_tensor("qk_spill", [H, 2, D, T], F32R)
    gate_spill = nc.dram_tensor("gate_spill", [16, 128, T], F32)
    attn_spill = nc.dram_tensor("attn_spill", [16, 128, T], F32R)
    c_dram = nc.dram_tensor("c_dram", [H, D, NBLK * D], F32)
    v_dram = nc.dram_tensor("v_dram", [NCH, 128, H * D], F32R)
    eloc = nc.dram_tensor("eloc", [H, D, D], F32)
    egath = nc.dram_tensor("egath", [NC, H, D, D], F32, addr_space="Shared")
    ssq_rt = nc.dram_tensor("ssq_rt", [T], F32)

    bd_f = [float(x) for x in _decay()[4]]

    with tile.TileContext(nc, pool_alloc_mode="stack") as tc:
        # ---- constants + resident tensors -------------------------------
        ident, free_ident = tc.tile([128, 128], F32, name="ident")
        make_identity(nc, ident[:])
        ones_f, free_ones_f = tc.tile([128, 1], F32, name="ones_f")
        nc.vector.memset(ones_f[:], 1.0)
        ones, free_ones = tc.tile([128, 1], F32R, name="ones")
        nc.scalar.copy(ones[:], ones_f[:])
        eps_sb, free_eps = tc.tile([128, 1], F32, name="eps_sb")
        nc.vector.memset(eps_sb[:], EPS)
        nw_sb, free_nw = tc.tile([128, 16], F32, name="nw_sb")
        nc.sync.dma_start(nw_sb[:], nw[:])
        kdm_sb, free_kdm = tc.tile([128, 2 * H], F32, name="kdm_sb")
        nc.sync.dma_start(kdm_sb[:], kdm[:])
        swm_sb, free_swm = tc.tile([D, H * NC], F32, name="swm_sb")
        nc.sync.dma_start(swm_sb[:], swm[:])

        _xt_ctx = ExitStack()
        xt_pool = _xt_ctx.enter_context(tc.tile_pool(name="xt_pool", bufs=1))
        xT = xt_pool.tile([128, 16, T], F32R, name="xT")
        for k in range(16):
            nc.sync.dma_start(xT[:, k, :], hsT[k * 128 : (k + 1) * 128, :])
        _v_ctx = ExitStack()
        v_pool = _v_ctx.enter_context(tc.tile_pool(name="v_pool", bufs=1))
        V_sb = v_pool.tile([128, NCH, H * D], F32R, name="V_sb")

        # ---- phase V: value projection (tok-major, all heads) -----------
        with tc.tile_pool(name="wv_p", bufs=3) as wv_p, tc.tile_pool(
            name="ps_v", bufs=1, space="PSUM"
        ) as ps_v:
            for n in range(4):
                pv = [
                    ps_v.tile([128, 512], F32, name=f"pv{m}") for m in range(NCH)
                ]
                for k in range(16):
                    wv_t = wv_p.tile([128, 512], F32R, name="wv_t")
                    nc.sync.dma_start(
                        wv_t[:], wvT[k * 128 : (k + 1) * 128, n * 512 : (n + 1) * 512]
                    )
                    for m in range(NCH):
                        nc.tensor.matmul(
                            pv[m][:],
                            xT[:, k, m * 128 : (m + 1) * 128],
                            wv_t[:],
                            start=(k == 0),
                            stop=(k == 15),
                        )
                for m in range(NCH):
                    nc.scalar.activation(
                        V_sb[:, m, n * 512 : (n + 1) * 512], pv[m][:], AF.Silu
                    )

        # ---- phase QK: q/k projection (dim-major per head) + contribs ---
        with tc.tile_pool(name="wqk_p", bufs=2) as wqk_p, tc.tile_pool(
            name="qk_p", bufs=2
        ) as qk_p, tc.tile_pool(name="tok_p", bufs=2) as tok_p, tc.tile_pool(
            name="ce_p", bufs=2
        ) as ce_p, tc.tile_pool(
            name="ps_qk", bufs=2, space="PSUM"
        ) as ps_qk, tc.tile_pool(
            name="ps_t", bufs=2, space="PSUM"
        ) as ps_t, tc.tile_pool(
            name="ps_c", bufs=2, space="PSUM"
        ) as ps_c:
            for h in range(H):
                wqk_t = wqk_p.tile([128, 16, 128], F32R, name="wqk_t")
                nc.sync.dma_start(
                    wqk_t[:],
                    wqkT[:, h * 128 : (h + 1) * 128].rearrange(
                        "(ko p) m -> p ko m", p=128
                    ),
                )
                pqk = ps_qk.tile([128, 2, 512], F32, name="pqk")
                for n in range(2):
                    for k in range(16):
                        nc.tensor.matmul(
                            pqk[:, n, :],
                            wqk_t[:, k, :],
                            xT[:, k, n * 512 : (n + 1) * 512],
                            start=(k == 0),
                            stop=(k == 15),
                        )
                qT_t = qk_p.tile([D, T], F32R, name="qT_t")
                kT_t = qk_p.tile([D, T], F32R, name="kT_t")
                nc.scalar.activation(
                    qT_t[:], pqk[0:D].rearrange("p n f -> p (n f)"), AF.Silu
                )
                nc.scalar.activation(
                    kT_t[:], pqk[D:128].rearrange("p n f -> p (n f)"), AF.Silu
                )
                nc.sync.dma_start(qk_spill[h, 0], qT_t[:])
                nc.sync.dma_start(qk_spill[h, 1], kT_t[:])

                # k back to tok-major via PE transpose
                k_tok = tok_p.tile([128, NCH, D], F32R, name="k_tok")
                for m in range(NCH):
                    pst = ps_t.tile([128, D], F32, name="pst")
                    nc.tensor.transpose(
                        pst[:],
                        kT_t[:, m * 128 : (m + 1) * 128].bitcast(F32),
                        ident[0:D, 0:D],
                    )
                    nc.scalar.copy(k_tok[:, m, :], pst[:])
                # v scaled by key-decay
                v_kd = tok_p.tile([128, NCH, D], F32R, name="v_kd")
                for m in range(NCH):
                    nc.vector.tensor_scalar_mul(
                        v_kd[:, m, :],
                        V_sb[:, m, h * D : (h + 1) * D],
                        kdm_sb[:, 2 * h + (m % 2) : 2 * h + (m % 2) + 1],
                    )
                # block contributions C_jb = (k*kd)^T v and chunk summary E
                c_st = ce_p.tile([D, NBLK, D], F32, name="c_st")
                for jb in range(NBLK):
                    pc = ps_c.tile([D, D], F32, name="pc")
                    for half in range(2):
                        m = 2 * jb + half
                        nc.tensor.matmul(
                            pc[:],
                            k_tok[:, m, :],
                            v_kd[:, m, :],
                            start=(half == 0),
                            stop=(half == 1),
                        )
                    nc.scalar.copy(c_st[:, jb, :], pc[:])
                nc.sync.dma_start(c_dram[h], c_st[:].rearrange("d b e -> d (b e)"))
                e_t = ce_p.tile([D, D], F32, name="e_t")
                nc.vector.tensor_copy(e_t[:], c_st[:, 0, :])
                for jb in range(1, NBLK):
                    nc.vector.scalar_tensor_tensor(
                        e_t[:], e_t[:], bd_f[h], c_st[:, jb, :], ALU.mult, ALU.add
                    )
                nc.sync.dma_start(eloc[h], e_t[:])

        for m in range(NCH):
            nc.sync.dma_start(v_dram[m], V_sb[:, m, :])
        _v_ctx.close()

        # ---- collective: share per-chunk KV summaries -------------------
        nc.gpsimd.collective_compute(
            "AllGather",
            ALU.bypass,
            replica_groups=[list(range(NC))],
            ins=[eloc[:]],
            outs=[egath[:]],
        )

        # ---- phase A: attention per head --------------------------------
        with tc.tile_pool(name="aq_p", bufs=2) as aq_p, tc.tile_pool(
            name="am_p", bufs=2
        ) as am_p, tc.tile_pool(name="ss_p", bufs=3) as ss_p, tc.tile_pool(
            name="ys_p", bufs=3
        ) as ys_p, tc.tile_pool(name="vh_p", bufs=2) as vh_p, tc.tile_pool(
            name="gw_p", bufs=2
        ) as gw_p, tc.tile_pool(name="go_p", bufs=2) as go_p, tc.tile_pool(
            name="ps_aw", bufs=2, space="PSUM"
        ) as ps_aw, tc.tile_pool(
            name="ps_ys", bufs=2, space="PSUM"
        ) as ps_ys, tc.tile_pool(
            name="ps_g", bufs=2, space="PSUM"
        ) as ps_g:
            for h in range(H):
                qT_a = aq_p.tile([D, T], F32R, name="qT_a")
                kT_a = aq_p.tile([D, T], F32R, name="kT_a")
                nc.sync.dma_start(qT_a[:], qk_spill[h, 0])
                nc.sync.dma_start(kT_a[:], qk_spill[h, 1])
                dd_t = am_p.tile([128, 2, BS], F32, name="dd_t")
                nc.sync.dma_start(dd_t[:], ddm[h].rearrange("c p i -> p c i"))
                qd_t = am_p.tile([D, BS], F32, name="qd_t")
                nc.sync.dma_start(qd_t[:], qdm[h])
                eg_t = am_p.tile([D, NC, D], F32, name="eg_t")
                nc.sync.dma_start(eg_t[:], egath[:, h, :, :].rearrange("c d e -> d c e"))
                c_a = am_p.tile([D, NBLK, D], F32, name="c_a")
                nc.sync.dma_start(c_a[:], c_dram[h].rearrange("d (b e) -> d b e", b=NBLK))
                v_h = vh_p.tile([128, NCH, D], F32R, name="v_h")
                nc.sync.dma_start(
                    v_h[:],
                    v_dram[:, :, h * D : (h + 1) * D].rearrange("m p e -> p m e"),
                )

                qdq = aq_p.tile([D, NBLK, BS], F32R, name="qdq")
                for jb in range(NBLK):
                    nc.vector.tensor_mul(
                        qdq[:, jb, :],
                        qT_a[:, jb * BS : (jb + 1) * BS].bitcast(F32),
                        qd_t[:],
                    )
                ss = ss_p.tile([D, D], F32R, name="ss")
                nc.vector.tensor_scalar_mul(
                    ss[:], eg_t[:, 0, :], swm_sb[:, h * NC : h * NC + 1]
                )
                for cc in range(1, NC):
                    nc.vector.scalar_tensor_tensor(
                        ss[:],
                        eg_t[:, cc, :],
                        swm_sb[:, h * NC + cc : h * NC + cc + 1],
                        ss[:],
                        ALU.mult,
                        ALU.add,
                    )
                ys_st = ys_p.tile([D, NBLK, BS], F32R, name="ys_st")
                for jb in range(NBLK):
                    paw = ps_aw.tile([128, 2, BS], F32, name="paw")
                    for jc in range(2):
                        nc.tensor.matmul(
                            paw[:, jc, :],
                            kT_a[:, jb * BS + jc * 128 : jb * BS + (jc + 1) * 128],
                            qT_a[:, jb * BS : (jb + 1) * BS],
                            start=True,
                            stop=True,
                        )
                    awm = ys_p.tile([128, 2, BS], F32R, name="awm")
                    nc.vector.tensor_mul(awm[:], paw[:], dd_t[:])
                    pys = ps_ys.tile([D, BS], F32, name="pys")
                    nc.tensor.matmul(
                        pys[:], ss[:], qdq[:, jb, :], start=True, stop=False
                    )
                    for jc in range(2):
                        nc.tensor.matmul(
                            pys[:],
                            v_h[:, 2 * jb + jc, :],
                            awm[:, jc, :],
                            start=False,
                            stop=(jc == 1),
                        )
                    nc.scalar.copy(ys_st[:, jb, :], pys[:])
                    if jb < NBLK - 1:
                        ss2 = ss_p.tile([D, D], F32R, name="ss")
                        nc.vector.scalar_tensor_tensor(
                            ss2[:], ss[:], bd_f[h], c_a[:, jb, :], ALU.mult, ALU.add
                        )
                        ss = ss2
                nc.sync.dma_start(
                    attn_spill[h // 2, (h % 2) * D : (h % 2 + 1) * D, :],
                    ys_st[:].rearrange("d b i -> d (b i)"),
                )
                # interleaved gate-projection chunk: keeps PE warm+dense
                if h % 2 == 1:
                    gm = h // 2
                    gw_t = gw_p.tile([128, 16, 128], F32R, name="gw_t")
                    nc.sync.dma_start(
                        gw_t[:],
                        gwT[:, gm * 128 : (gm + 1) * 128].rearrange(
                            "(ko p) g -> p ko g", p=128
                        ),
                    )
                    for gn in range(2):
                        pg = ps_g.tile([128, 512], F32, name="pg")
                        for gk in range(16):
                            nc.tensor.matmul(
                                pg[:],
                                gw_t[:, gk, :],
                                xT[:, gk, gn * 512 : (gn + 1) * 512],
                                start=(gk == 0),
                                stop=(gk == 15),
                            )
                        go_t = go_p.tile([128, 512], F32, name="go_t")
                        nc.scalar.activation(go_t[:], pg[:], AF.Sigmoid)
                        nc.sync.dma_start(
                            gate_spill[gm, :, gn * 512 : (gn + 1) * 512], go_t[:]
                        )
        _xt_ctx.close()

        # ---- phase F: rmsnorm + gate + output projection ----------------
        _g_ctx = ExitStack()
        g_pool = _g_ctx.enter_context(tc.tile_pool(name="g_pool", bufs=1))
        gate_sb = g_pool.tile([128, 16, T], F32, name="gate_sb")
        for c in range(16):
            nc.sync.dma_start(gate_sb[:, c, :], gate_spill[c])
        with tc.tile_pool(name="sq_p", bufs=2) as sq_p, tc.tile_pool(
            name="an_p", bufs=3
        ) as an_p:
          with tc.tile_pool(name="ps_sq", bufs=1, space="PSUM") as ps_sq:
            ssq0 = ps_sq.tile([1, 512], F32, name="ssq0")
            ssq1 = ps_sq.tile([1, 512], F32, name="ssq1")
            for c in range(16):
                at = an_p.tile([128, T], F32R, name="at")
                nc.sync.dma_start(at[:], attn_spill[c])
                sq = sq_p.tile([128, T], F32R, name="sq")
                nc.scalar.activation(sq[:], at[:].bitcast(F32), AF.Square)
                for half in range(2):
                    nc.tensor.matmul(
                        [ssq0, ssq1][half][:],
                        ones[:],
                        sq[:, half * 512 : (half + 1) * 512],
                        start=(c == 0),
                        stop=(c == 15),
                        skip_group_check=True,
                    )
                nc.vector.scalar_tensor_tensor(
                    gate_sb[:, c, :].bitcast(F32R),
                    at[:].bitcast(F32),
                    nw_sb[:, c : c + 1],
                    gate_sb[:, c, :],
                    ALU.mult,
                    ALU.mult,
                )
            ssq_sb = sq_p.tile([1, T], F32, name="ssq_sb")
            nc.vector.tensor_copy(ssq_sb[:, 0:512], ssq0[:])
            nc.vector.tensor_copy(ssq_sb[:, 512:1024], ssq1[:])
            nc.sync.dma_start(ssq_rt[:], ssq_sb[:])
          if True:
            ns_l = sq_p.tile([128, NCH], F32, name="ns_l")
            nc.sync.dma_start(ns_l[:], ssq_rt.rearrange("(c p) -> p c", p=128))
            ns_t = sq_p.tile([128, NCH], F32, name="ns_t")
            nc.scalar.activation(
                ns_t[:], ns_l[:], AF.Sqrt, bias=eps_sb[:, 0:1], scale=1.0 / (H * D)
            )
            ns_sb = sq_p.tile([128, NCH], F32, name="ns_sb")
            nc.vector.reciprocal(ns_sb[:], ns_t[:])

            with tc.tile_pool(name="ow_p", bufs=3) as ow_p, tc.tile_pool(
                name="oo_p", bufs=3
            ) as oo_p, tc.tile_pool(name="ps_o", bufs=1, space="PSUM") as ps_o:
                for n in range(4):
                    po = [
                        ps_o.tile([128, 512], F32, name=f"po{m}") for m in range(NCH)
                    ]
                    for k in range(16):
                        ow_t = ow_p.tile([128, 512], F32R, name="ow_t")
                        nc.sync.dma_start(
                            ow_t[:],
                            owT[k * 128 : (k + 1) * 128, n * 512 : (n + 1) * 512],
                        )
                        for m in range(NCH):
                            nc.tensor.matmul(
                                po[m][:],
                                gate_sb[:, k, m * 128 : (m + 1) * 128].bitcast(F32R),
                                ow_t[:],
                                start=(k == 0),
                                stop=(k == 15),
                            )
                    for m in range(NCH):
                        oo_t = oo_p.tile([128, 512], F32, name="oo_t")
                        nc.scalar.mul(oo_t[:], po[m][:], ns_sb[:, m : m + 1])
                        nc.sync.dma_start(
                            out[m * 128 : (m + 1) * 128, n * 512 : (n + 1) * 512],
                            oo_t[:],
                        )
        _g_ctx.close()
        free_swm()
        free_kdm()
        free_nw()
        free_eps()
        free_ones()
        free_ones_f()
        free_ident()
    nc.finalize()
    return nc


_CACHE = {}


def _get_nc():
    if "nc" not in _CACHE:
        _CACHE["nc"] = _build_nc()
    return _CACHE["nc"]


def _host_prep(hidden_states, qkv_w, out_w, gate_w, norm_w):
    slope, qd, kd, dd, bd = _decay()
    w3 = qkv_w.reshape(H, 3 * D, HID)
    wq = w3[:, 0:D, :]
    wk = w3[:, D : 2 * D, :]
    wv = w3[:, 2 * D : 3 * D, :]
    wqk = np.concatenate([wq, wk], axis=1).reshape(2 * H * D, HID)
    wqkT = np.ascontiguousarray(wqk.T, dtype=np.float32)
    wvT = np.ascontiguousarray(wv.reshape(H * D, HID).T, dtype=np.float32)
    gwT = np.ascontiguousarray(gate_w.T, dtype=np.float32)
    owT = np.ascontiguousarray(out_w.T, dtype=np.float32)
    ddm = np.ascontiguousarray(
        dd.transpose(0, 2, 1).reshape(H, 2, 128, BS), dtype=np.float32
    )
    qdm = np.ascontiguousarray(
        np.broadcast_to(qd[:, None, :], (H, D, BS)), dtype=np.float32
    )
    kdm = np.ascontiguousarray(
        kd.reshape(H, 2, 128).transpose(2, 0, 1).reshape(128, 2 * H), dtype=np.float32
    )
    nw = np.ascontiguousarray(norm_w.reshape(16, 128).T, dtype=np.float32)

    shared = dict(wqkT=wqkT, wvT=wvT, gwT=gwT, owT=owT, ddm=ddm, qdm=qdm,
                  kdm=kdm, nw=nw)
    in_maps = []
    for c in range(NC):
        bb, p = c // 4, c % 4
        hsT = np.ascontiguousarray(
            hidden_states[bb, p * T : (p + 1) * T, :].T, dtype=np.float32
        )
        sw = np.zeros((H, NC), dtype=np.float64)
        for cc in range(NC):
            if cc // 4 == bb and cc % 4 < p:
                sw[:, cc] = bd ** (4.0 * (p - 1 - (cc % 4)))
        swm = np.ascontiguousarray(
            np.broadcast_to(sw.reshape(1, H * NC), (D, H * NC)), dtype=np.float32
        )
        in_maps.append(dict(hsT=hsT, swm=swm, **shared))
    return in_maps


def _run(inputs, trace=False):
    nc = _get_nc()
    in_maps = _host_prep(
        np.asarray(inputs["hidden_states"], dtype=np.float32),
        np.asarray(inputs["qkv_w"], dtype=np.float32),
        np.asarray(inputs["out_w"], dtype=np.float32),
        np.asarray(inputs["gate_w"], dtype=np.float32),
        np.asarray(inputs["norm_w"], dtype=np.float32),
    )
    res = run_bass_kernel_spmd(nc, in_maps, core_ids=list(range(NC)), trace=trace)
    full = np.empty((B, S, HID), dtype=np.float32)
    for c in range(NC):
        bb, p = c // 4, c % 4
        full[bb, p * T : (p + 1) * T, :] = res.results[c]["out"]
    return full, res


def kernel(**inputs):
    return _run(inputs, trace=False)[0]


def kernel_traced(**inputs):
    full, res = _run(inputs, trace=True)
    return full, res.exec_time_ns



# revision 12
# speedup vs baseline: 1.3861x; 1.3861x over previous
"""MiniMax Lightning Attention on 8 Trainium2 NeuronCores.

Sharding: sequence-parallel. Core c handles batch c//4, token chunk
(c%4)*1024..+1024 (4 blocks of 256). The per-block decay-state recurrence
crosses chunk boundaries; each core computes its local per-chunk decay-
weighted KV summary E, an AllGather shares the 8 summaries, and each core
reconstructs its chunk-start state as a decay-weighted sum.

Fused single-residency design: all big activations (x, V, q, ys, gate)
stay SBUF-resident in bf16; weights stream in bf16. Heads are processed
in pairs (dim groups of 128) so projections, transposes, and PSUM tiles
use full 128-partition width. Intra-block attention runs in pass 1
(before the collective); the cross-chunk "inter" term is a cheap pass 2
after the AllGather, hidden behind the gate projection.
"""

import numpy as np
import ml_dtypes

from contextlib import ExitStack

import concourse.bacc as bacc
import concourse.mybir as mybir
import concourse.tile as tile
from concourse.bass_utils import run_bass_kernel_spmd
from concourse.masks import make_identity

AF = mybir.ActivationFunctionType
ALU = mybir.AluOpType
F32 = mybir.dt.float32
F32R = mybir.dt.float32r
BF16 = mybir.dt.bfloat16

H = 32
D = 64
BS = 256
HID = 2048
B = 2
S = 4096
NC = 8
T = S // 4            # tokens per core (1024)
NCH = T // 128        # 8 token chunks of 128
NBLK = T // BS        # 4 blocks per core
G = H // 2            # 16 head pairs (dim groups of 128)
KC = HID // 128       # 16 contraction chunks
LAYER_IDX = 0
NUM_LAYERS = 32
EPS = 1e-5

BF = ml_dtypes.bfloat16


def _decay():
    base = 1.0 / 2.0 ** (8.0 / H)
    rate = base ** (np.arange(H, dtype=np.float64) + 1.0)
    factor = 1.0 - LAYER_IDX / (NUM_LAYERS - 1 + 1e-5) + 1e-5
    slope = rate * factor                                  # (H,)
    r = np.arange(BS, dtype=np.float64) + 1.0
    qd = np.exp(-slope[:, None] * r[None, :])              # (H, BS) query decay
    kd = np.exp(-slope[:, None] * (BS - r[None, :]))       # (H, BS) key decay
    ij = r[:, None] - r[None, :]                           # i - j
    dd = np.where(
        ij[None] >= 0, np.exp(-slope[:, None, None] * ij[None]), 0.0
    )                                                      # (H, BS_i, BS_j)
    bd = np.exp(-slope * BS)                               # (H,) block decay
    return slope, qd, kd, dd, bd


def _build_nc():
    nc = bacc.Bacc(num_devices=NC)
    hsT = nc.declare_dram_parameter("hsT", [HID, T], BF16, isOutput=False)
    wvT = nc.declare_dram_parameter("wvT", [HID, H * D], BF16, isOutput=False)
    wqkT = nc.declare_dram_parameter("wqkT", [HID, G, 256], BF16, isOutput=False)
    gwT = nc.declare_dram_parameter("gwT", [HID, HID], BF16, isOutput=False)
    owT = nc.declare_dram_parameter("owT", [H * D, HID], BF16, isOutput=False)
    ddm = nc.declare_dram_parameter("ddm", [G, 128, 1024], BF16, isOutput=False)
    qdm = nc.declare_dram_parameter("qdm", [128, G, BS], BF16, isOutput=False)
    kdm = nc.declare_dram_parameter("kdm", [128, 2 * H], F32, isOutput=False)
    bdm = nc.declare_dram_parameter("bdm", [128, G], F32, isOutput=False)
    swm = nc.declare_dram_parameter("swm", [128, G * NC], F32, isOutput=False)
    nw = nc.declare_dram_parameter("nw", [128, 16], F32, isOutput=False)
    out = nc.declare_dram_parameter("out", [T, HID], F32, isOutput=True)

    eloc = nc.dram_tensor("eloc", [2, G, D, D], BF16)
    egath = nc.dram_tensor("egath", [NC, 2, G, D, D], BF16, addr_space="Shared")
    ssq_rt = nc.dram_tensor("ssq_rt", [T], F32)

    with tile.TileContext(nc, pool_alloc_mode="stack") as tc:
        # ---- constants -------------------------------------------------
        _c_ctx = ExitStack()
        c_pool = _c_ctx.enter_context(tc.tile_pool(name="c_pool", bufs=1))
        ident = c_pool.tile([128, 128], BF16, name="ident")
        make_identity(nc, ident[:])
        ones = c_pool.tile([128, 1], BF16, name="ones")
        nc.vector.memset(ones[:], 1.0)
        eps_sb = c_pool.tile([128, 1], F32, name="eps_sb")
        nc.vector.memset(eps_sb[:], EPS)
        kdm_sb = c_pool.tile([128, 2 * H], F32, name="kdm_sb")
        nc.sync.dma_start(kdm_sb[:], kdm[:])
        bdm_sb = c_pool.tile([128, G], F32, name="bdm_sb")
        nc.sync.dma_start(bdm_sb[:], bdm[:])
        swm_sb = c_pool.tile([128, G * NC], F32, name="swm_sb")
        nc.sync.dma_start(swm_sb[:], swm[:])
        nw_sb = c_pool.tile([128, 16], F32, name="nw_sb")
        nc.sync.dma_start(nw_sb[:], nw[:])

        # ---- persistent activations ------------------------------------
        _xt_ctx = ExitStack()
        xt_pool = _xt_ctx.enter_context(tc.tile_pool(name="xt_pool", bufs=1))
        xT = xt_pool.tile([128, KC, T], BF16, name="xT")
        for q4 in range(4):
            nc.sync.dma_start(
                xT[:, q4 * 4 : (q4 + 1) * 4, :],
                hsT[q4 * 512 : (q4 + 1) * 512, :].rearrange(
                    "(k p) t -> p k t", p=128
                ),
            )
        _q_ctx = ExitStack()
        q_pool = _q_ctx.enter_context(tc.tile_pool(name="q_pool", bufs=1))
        qT_sb = q_pool.tile([128, G, T], BF16, name="qT_sb")
        _ys_ctx = ExitStack()
        ys_pool = _ys_ctx.enter_context(tc.tile_pool(name="ys_pool", bufs=1))
        ys_sb = ys_pool.tile([128, G, T], BF16, name="ys_sb")
        _ce_ctx = ExitStack()
        ce_pool = _ce_ctx.enter_context(tc.tile_pool(name="ce_pool", bufs=1))
        c_sb = ce_pool.tile([128, G, NBLK, D], F32, name="c_sb")
        E_sb = ce_pool.tile([128, G, D], BF16, name="E_sb")
        # v_pool opened last among persistents: it is the only one released
        # mid-stream (stack allocator frees LIFO only)
        _v_ctx = ExitStack()
        v_pool = _v_ctx.enter_context(tc.tile_pool(name="v_pool", bufs=1))
        V_sb = v_pool.tile([128, NCH, H * D], BF16, name="V_sb")

        # ---- phase V: value projection (tok-major, all heads) -----------
        with tc.tile_pool(name="wv_p", bufs=3) as wv_p, tc.tile_pool(
            name="ps_v", bufs=1, space="PSUM"
        ) as ps_v:
            for n in range(4):
                pv = [
                    ps_v.tile([128, 512], F32, name=f"pv{m}") for m in range(NCH)
                ]
                for k in range(KC):
                    wv_t = wv_p.tile([128, 512], BF16, name="wv_t")
                    nc.sync.dma_start(
                        wv_t[:], wvT[k * 128 : (k + 1) * 128, n * 512 : (n + 1) * 512]
                    )
                    for m in range(NCH):
                        nc.tensor.matmul(
                            pv[m][:],
                            xT[:, k, m * 128 : (m + 1) * 128],
                            wv_t[:],
                            start=(k == 0),
                            stop=(k == KC - 1),
                        )
                for m in range(NCH):
                    nc.scalar.activation(
                        V_sb[:, m, n * 512 : (n + 1) * 512], pv[m][:], AF.Silu
                    )

        # ---- pass 1: per head-pair: QK proj, C/E summary, intra attn ----
        with tc.tile_pool(name="wqk_p", bufs=3) as wqk_p, tc.tile_pool(
            name="dd_p", bufs=2
        ) as dd_p, tc.tile_pool(name="kt_p", bufs=2) as kt_p, tc.tile_pool(
            name="ktok_p", bufs=2
        ) as ktok_p, tc.tile_pool(name="vkd_p", bufs=2) as vkd_p, tc.tile_pool(
            name="awm_p", bufs=3
        ) as awm_p, tc.tile_pool(name="ps1", bufs=1, space="PSUM") as ps1:
            for g in range(G):
                wqk_a = wqk_p.tile([128, 8, 256], BF16, name="wqk_t")
                nc.sync.dma_start(
                    wqk_a[:],
                    wqkT[0:1024, g, :].rearrange("(ko p) c -> p ko c", p=128),
                )
                wqk_b = wqk_p.tile([128, 8, 256], BF16, name="wqk_t")
                nc.sync.dma_start(
                    wqk_b[:],
                    wqkT[1024:2048, g, :].rearrange("(ko p) c -> p ko c", p=128),
                )
                dd_g = dd_p.tile([128, 1024], BF16, name="dd_g")
                nc.sync.dma_start(dd_g[:], ddm[g])

                # q projection (dim-major, both heads: partitions 0:64/64:128)
                for half in range(2):
                    pq = ps1.tile([128, 512], F32, name="pq", tag="proj", bufs=2)
                    for k in range(KC):
                        wt = wqk_a if k < 8 else wqk_b
                        nc.tensor.matmul(
                            pq[:],
                            wt[:, k % 8, 0:128],
                            xT[:, k, half * 512 : (half + 1) * 512],
                            start=(k == 0),
                            stop=(k == KC - 1),
                        )
                    nc.scalar.activation(
                        qT_sb[:, g, half * 512 : (half + 1) * 512], pq[:], AF.Silu
                    )
                # k projection
                kT2 = kt_p.tile([128, T], BF16, name="kT2")
                for half in range(2):
                    pk = ps1.tile([128, 512], F32, name="pk", tag="proj", bufs=2)
                    for k in range(KC):
                        wt = wqk_a if k < 8 else wqk_b
                        nc.tensor.matmul(
                            pk[:],
                            wt[:, k % 8, 128:256],
                            xT[:, k, half * 512 : (half + 1) * 512],
                            start=(k == 0),
                            stop=(k == KC - 1),
                        )
                    nc.scalar.activation(
                        kT2[:, half * 512 : (half + 1) * 512], pk[:], AF.Silu
                    )

                # k -> tok-major via PE transpose (8x 128x128, batched by 4)
                k_tok = ktok_p.tile([128, NCH, 128], BF16, name="k_tok")
                for b2 in range(2):
                    ptr = ps1.tile([128, 4, 128], BF16, name="ptr", tag="sm", bufs=3)
                    for ch in range(4):
                        cc = b2 * 4 + ch
                        nc.tensor.matmul(
                            ptr[:, ch, :],
                            kT2[:, cc * 128 : (cc + 1) * 128],
                            ident[:],
                            is_transpose=True,
                            skip_group_check=True,
                        )
                    nc.scalar.copy(
                        k_tok[:, b2 * 4 : (b2 + 1) * 4, :].rearrange(
                            "p a b -> p (a b)"
                        ),
                        ptr[:].rearrange("p a b -> p (a b)"),
                    )

                # v scaled by key-decay
                v_kd = vkd_p.tile([128, NCH, 128], BF16, name="v_kd")
                for m in range(NCH):
                    for hh in range(2):
                        h = 2 * g + hh
                        nc.vector.tensor_scalar_mul(
                            v_kd[:, m, hh * 64 : (hh + 1) * 64],
                            V_sb[:, m, g * 128 + hh * 64 : g * 128 + (hh + 1) * 64],
                            kdm_sb[:, 2 * h + (m % 2) : 2 * h + (m % 2) + 1],
                        )

                # block contributions C_jb = (k*kd)^T v (both heads at once)
                pc2 = ps1.tile([128, 4, 128], F32, name="pc2", tag="sm", bufs=3)
                for jb in range(NBLK):
                    for jc in range(2):
                        m = 2 * jb + jc
                        nc.tensor.matmul(
                            pc2[:, jb, :],
                            k_tok[:, m, :],
                            v_kd[:, m, :],
                            start=(jc == 0),
                            stop=(jc == 1),
                            skip_group_check=True,
                        )
                for hh in range(2):
                    sl = slice(hh * 64, (hh + 1) * 64)
                    nc.scalar.copy(
                        c_sb[sl, g, :, :], pc2[sl, :, hh * 64 : (hh + 1) * 64]
                    )
                # chunk summary E = sum_jb bd^(3-jb) C_jb
                nc.vector.tensor_copy(E_sb[:, g, :], c_sb[:, g, 0, :])
                for jb in range(1, NBLK):
                    nc.vector.scalar_tensor_tensor(
                        E_sb[:, g, :],
                        E_sb[:, g, :],
                        bdm_sb[:, g : g + 1],
                        c_sb[:, g, jb, :],
                        ALU.mult,
                        ALU.add,
                    )

                # intra-block attention
                for jb in range(NBLK):
                    awms = []
                    for hh in range(2):
                        hs = slice(hh * 64, (hh + 1) * 64)
                        paw = ps1.tile([128, 2, 256], F32, name="paw", tag="aw", bufs=2)
                        for jc in range(2):
                            nc.tensor.matmul(
                                paw[:, jc, :],
                                kT2[hs, jb * 256 + jc * 128 : jb * 256 + (jc + 1) * 128],
                                qT_sb[hs, g, jb * 256 : (jb + 1) * 256],
                                start=True,
                                stop=True,
                                skip_group_check=True,
                            )
                        awm = awm_p.tile([128, 2, 256], BF16, name="awm")
                        nc.vector.tensor_mul(
                            awm[:].rearrange("p a b -> p (a b)"),
                            paw[:].rearrange("p a b -> p (a b)"),
                            dd_g[:, hh * 512 : (hh + 1) * 512],
                        )
                        awms.append(awm)
                    pys = ps1.tile([128, 256], F32, name="pys", tag="sm", bufs=3)
                    for hh in range(2):
                        for jc in range(2):
                            m = 2 * jb + jc
                            nc.tensor.matmul(
                                pys[hh * 64 : (hh + 1) * 64, :],
                                V_sb[:, m, g * 128 + hh * 64 : g * 128 + (hh + 1) * 64],
                                awms[hh][:, jc, :],
                                start=(jc == 0),
                                stop=(jc == 1),
                                skip_group_check=True,
                            )
                    nc.scalar.copy(ys_sb[:, g, jb * 256 : (jb + 1) * 256], pys[:])

        # ---- collective: share per-chunk KV summaries -------------------
        nc.sync.dma_start(eloc[0].rearrange("g d e -> d g e"), E_sb[0:64, :, :])
        nc.sync.dma_start(eloc[1].rearrange("g d e -> d g e"), E_sb[64:128, :, :])
        nc.gpsimd.collective_compute(
            "AllGather",
            ALU.bypass,
            replica_groups=[list(range(NC))],
            ins=[eloc[:]],
            outs=[egath[:]],
        )
        _v_ctx.close()

        # ---- gate projection (overlaps the collective) ------------------
        _g_ctx = ExitStack()
        g_pool = _g_ctx.enter_context(tc.tile_pool(name="g_pool", bufs=1))
        gate_sb = g_pool.tile([128, G, T], BF16, name="gate_sb")
        _eg_ctx = ExitStack()
        eg_pool = _eg_ctx.enter_context(tc.tile_pool(name="eg_pool", bufs=1))
        EG = eg_pool.tile([128, G, NC, D], BF16, name="EG")
        qd_sb = eg_pool.tile([128, G, BS], BF16, name="qd_sb")
        nc.sync.dma_start(qd_sb[:], qdm[:])
        for cc in range(NC):
            nc.sync.dma_start(
                EG[0:64, :, cc, :], egath[cc, 0].rearrange("g d e -> d g e")
            )
            nc.sync.dma_start(
                EG[64:128, :, cc, :], egath[cc, 1].rearrange("g d e -> d g e")
            )

        with tc.tile_pool(name="gw_p", bufs=3) as gw_p, tc.tile_pool(
            name="ps_g", bufs=1, space="PSUM"
        ) as ps_g:
            for gm in range(16):
                gw_t = gw_p.tile([128, KC, 128], BF16, name="gw_t")
                nc.sync.dma_start(
                    gw_t[:],
                    gwT[:, gm * 128 : (gm + 1) * 128].rearrange(
                        "(ko p) c -> p ko c", p=128
                    ),
                )
                for gn in range(2):
                    pg = ps_g.tile([128, 512], F32, name="pg", bufs=2)
                    for k in range(KC):
                        nc.tensor.matmul(
                            pg[:],
                            gw_t[:, k, :],
                            xT[:, k, gn * 512 : (gn + 1) * 512],
                            start=(k == 0),
                            stop=(k == KC - 1),
                        )
                    nc.scalar.activation(
                        gate_sb[:, gm, gn * 512 : (gn + 1) * 512], pg[:], AF.Sigmoid
                    )

        # ---- pass 2: cross-chunk inter attention ------------------------
        with tc.tile_pool(name="qdq_p", bufs=2) as qdq_p, tc.tile_pool(
            name="ss_p", bufs=2
        ) as ss_p, tc.tile_pool(name="ps2", bufs=1, space="PSUM") as ps2:
            for g in range(G):
                ss = ss_p.tile([128, D], BF16, name="ss")
                nc.vector.tensor_scalar_mul(
                    ss[:], EG[:, g, 0, :], swm_sb[:, g * NC : g * NC + 1]
                )
                for cc in range(1, NC):
                    nc.vector.scalar_tensor_tensor(
                        ss[:],
                        EG[:, g, cc, :],
                        swm_sb[:, g * NC + cc : g * NC + cc + 1],
                        ss[:],
                        ALU.mult,
                        ALU.add,
                    )
                for jb in range(NBLK):
                    qdq = qdq_p.tile([128, BS], BF16, name="qdq")
                    nc.vector.tensor_mul(
                        qdq[:], qT_sb[:, g, jb * 256 : (jb + 1) * 256], qd_sb[:, g, :]
                    )
                    pin = ps2.tile([128, BS], F32, name="pin", bufs=2)
                    for hh in range(2):
                        hs = slice(hh * 64, (hh + 1) * 64)
                        nc.tensor.matmul(
                            pin[hs, :],
                            ss[hs, :],
                            qdq[hs, :],
                            start=True,
                            stop=True,
                            skip_group_check=True,
                        )
                    nc.vector.tensor_add(
                        ys_sb[:, g, jb * 256 : (jb + 1) * 256],
                        ys_sb[:, g, jb * 256 : (jb + 1) * 256],
                        pin[:],
                    )
                    if jb < NBLK - 1:
                        ss2 = ss_p.tile([128, D], BF16, name="ss")
                        nc.vector.scalar_tensor_tensor(
                            ss2[:],
                            ss[:],
                            bdm_sb[:, g : g + 1],
                            c_sb[:, g, jb, :],
                            ALU.mult,
                            ALU.add,
                        )
                        ss = ss2

        # ---- phase F: rmsnorm + gate + output projection ----------------
        with tc.tile_pool(name="sq_p", bufs=2) as sq_p:
            with tc.tile_pool(name="ps_sq", bufs=1, space="PSUM") as ps_sq:
                ssq0 = ps_sq.tile([1, 512], F32, name="ssq0")
                ssq1 = ps_sq.tile([1, 512], F32, name="ssq1")
                for c in range(16):
                    sq = sq_p.tile([128, T], BF16, name="sq")
                    nc.scalar.activation(sq[:], ys_sb[:, c, :], AF.Square)
                    for half in range(2):
                        nc.tensor.matmul(
                            [ssq0, ssq1][half][:],
                            ones[:],
                            sq[:, half * 512 : (half + 1) * 512],
                            start=(c == 0),
                            stop=(c == 15),
                            skip_group_check=True,
                        )
                    nc.vector.scalar_tensor_tensor(
                        gate_sb[:, c, :],
                        ys_sb[:, c, :],
                        nw_sb[:, c : c + 1],
                        gate_sb[:, c, :],
                        ALU.mult,
                        ALU.mult,
                    )
                ssq_sb = sq_p.tile([1, T], F32, name="ssq_sb")
                nc.vector.tensor_copy(ssq_sb[:, 0:512], ssq0[:])
                nc.vector.tensor_copy(ssq_sb[:, 512:1024], ssq1[:])
                nc.sync.dma_start(ssq_rt[:], ssq_sb[:])
            ns_l = sq_p.tile([128, NCH], F32, name="ns_l")
            nc.sync.dma_start(ns_l[:], ssq_rt.rearrange("(c p) -> p c", p=128))
            ns_t = sq_p.tile([128, NCH], F32, name="ns_t")
            nc.scalar.activation(
                ns_t[:], ns_l[:], AF.Sqrt, bias=eps_sb[:, 0:1], scale=1.0 / (H * D)
            )
            ns_sb = sq_p.tile([128, NCH], F32, name="ns_sb")
            nc.vector.reciprocal(ns_sb[:], ns_t[:])

            with tc.tile_pool(name="ow_p", bufs=3) as ow_p, tc.tile_pool(
                name="oo_p", bufs=3
            ) as oo_p, tc.tile_pool(name="ps_o", bufs=1, space="PSUM") as ps_o:
                for n in range(4):
                    po = [
                        ps_o.tile([128, 512], F32, name=f"po{m}") for m in range(NCH)
                    ]
                    for k in range(KC):
                        ow_t = ow_p.tile([128, 512], BF16, name="ow_t")
                        nc.sync.dma_start(
                            ow_t[:],
                            owT[k * 128 : (k + 1) * 128, n * 512 : (n + 1) * 512],
                        )
                        for m in range(NCH):
                            nc.tensor.matmul(
                                po[m][:],
                                gate_sb[:, k, m * 128 : (m + 1) * 128],
                                ow_t[:],
                                start=(k == 0),
                                stop=(k == KC - 1),
                            )
                    for m in range(NCH):
                        oo_t = oo_p.tile([128, 512], F32, name="oo_t")
                        nc.scalar.mul(oo_t[:], po[m][:], ns_sb[:, m : m + 1])
                        nc.sync.dma_start(
                            out[m * 128 : (m + 1) * 128, n * 512 : (n + 1) * 512],
                            oo_t[:],
                        )
        _eg_ctx.close()
        _g_ctx.close()
        _ce_ctx.close()
        _ys_ctx.close()
        _q_ctx.close()
        _xt_ctx.close()
        _c_ctx.close()
    nc.finalize()
    return nc


_CACHE = {}


def _get_nc():
    if "nc" not in _CACHE:
        _CACHE["nc"] = _build_nc()
    return _CACHE["nc"]


def _host_prep(hidden_states, qkv_w, out_w, gate_w, norm_w):
    slope, qd, kd, dd, bd = _decay()
    w3 = qkv_w.reshape(H, 3 * D, HID)
    wq = w3[:, 0:D, :].reshape(H * D, HID)
    wk = w3[:, D : 2 * D, :].reshape(H * D, HID)
    wv = w3[:, 2 * D : 3 * D, :].reshape(H * D, HID)
    # wqkT[:, g, 0:128] = q dims of heads 2g,2g+1; [:, g, 128:256] = k dims
    wqkT = np.concatenate(
        [
            np.ascontiguousarray(wq.T).reshape(HID, G, 128),
            np.ascontiguousarray(wk.T).reshape(HID, G, 128),
        ],
        axis=2,
    ).astype(BF)
    wvT = np.ascontiguousarray(wv.T).astype(BF)
    gwT = np.ascontiguousarray(gate_w.T).astype(BF)
    owT = np.ascontiguousarray(out_w.T).astype(BF)

    # ddm[g, p, h*512 + jc*256 + i] = dd[2g+h, i, jc*128+p]
    dd_t = dd.transpose(0, 2, 1)  # (H, j, i)
    ddm = np.ascontiguousarray(
        dd_t.reshape(G, 2, 2, 128, BS).transpose(0, 3, 1, 2, 4).reshape(G, 128, 1024)
    ).astype(BF)
    # qdm[p, g, i] = qd[2g + p//64, i]
    qdm = np.ascontiguousarray(
        np.broadcast_to(qd.reshape(G, 2, 1, BS), (G, 2, 64, BS))
        .transpose(1, 2, 0, 3)
        .reshape(128, G, BS)
    ).astype(BF)
    # kdm[p, 2h+parity] = kd[h, parity*128+p]
    kdm = np.ascontiguousarray(
        kd.reshape(H, 2, 128).transpose(2, 0, 1).reshape(128, 2 * H)
    ).astype(np.float32)
    # bdm[p, g] = bd[2g + p//64]
    bdm = np.ascontiguousarray(
        np.broadcast_to(bd.reshape(G, 2, 1), (G, 2, 64)).transpose(1, 2, 0).reshape(128, G)
    ).astype(np.float32)
    nw = np.ascontiguousarray(norm_w.reshape(16, 128).T).astype(np.float32)

    shared = dict(wqkT=wqkT, wvT=wvT, gwT=gwT, owT=owT, ddm=ddm, qdm=qdm,
                  kdm=kdm, bdm=bdm, nw=nw)
    in_maps = []
    for c in range(NC):
        bb, p = c // 4, c % 4
        hsT = np.ascontiguousarray(
            hidden_states[bb, p * T : (p + 1) * T, :].T
        ).astype(BF)
        sw = np.zeros((H, NC), dtype=np.float64)
        for cc in range(NC):
            if cc // 4 == bb and cc % 4 < p:
                sw[:, cc] = bd ** (4.0 * (p - 1 - (cc % 4)))
        # swm[p_, g*8+cc] = sw[2g + p_//64, cc]
        swm = np.ascontiguousarray(
            np.broadcast_to(sw.reshape(G, 2, 1, NC), (G, 2, 64, NC))
            .transpose(1, 2, 0, 3)
            .reshape(128, G * NC)
        ).astype(np.float32)
        in_maps.append(dict(hsT=hsT, swm=swm, **shared))
    return in_maps


def _run(inputs, trace=False):
    nc = _get_nc()
    in_maps = _host_prep(
        np.asarray(inputs["hidden_states"], dtype=np.float32),
        np.asarray(inputs["qkv_w"], dtype=np.float32),
        np.asarray(inputs["out_w"], dtype=np.float32),
        np.asarray(inputs["gate_w"], dtype=np.float32),
        np.asarray(inputs["norm_w"], dtype=np.float32),
    )
    res = run_bass_kernel_spmd(nc, in_maps, core_ids=list(range(NC)), trace=trace)
    full = np.empty((B, S, HID), dtype=np.float32)
    for c in range(NC):
        bb, p = c // 4, c % 4
        full[bb, p * T : (p + 1) * T, :] = res.results[c]["out"]
    return full, res


def kernel(**inputs):
    return _run(inputs, trace=False)[0]


def kernel_traced(**inputs):
    full, res = _run(inputs, trace=True)
    return full, res.exec_time_ns


# revision 20
# speedup vs baseline: 1.4926x; 1.0769x over previous
"""MiniMax Lightning Attention on 8 Trainium2 NeuronCores.

Sharding: sequence-parallel. Core c handles batch c//4, token chunk
(c%4)*1024..+1024 (4 blocks of 256). The per-block decay-state recurrence
crosses chunk boundaries; each core computes its local per-chunk decay-
weighted KV summary E, an AllGather shares the 8 summaries, and each core
reconstructs its chunk-start state as a decay-weighted sum.

Fused single-residency design: all big activations (x, V, q, ys, gate)
stay SBUF-resident in bf16; weights stream in bf16. Heads are processed
in pairs (dim groups of 128) so projections, transposes, and PSUM tiles
use full 128-partition width. Intra-block attention runs in pass 1
(before the collective); the cross-chunk "inter" term is a cheap pass 2
after the AllGather, hidden behind the gate projection.
"""

import numpy as np
import ml_dtypes

from contextlib import ExitStack

import concourse.bacc as bacc
import concourse.mybir as mybir
import concourse.tile as tile
from concourse.bass_utils import run_bass_kernel_spmd
from concourse.masks import make_identity

AF = mybir.ActivationFunctionType
ALU = mybir.AluOpType
F32 = mybir.dt.float32
F32R = mybir.dt.float32r
BF16 = mybir.dt.bfloat16

H = 32
D = 64
BS = 256
HID = 2048
B = 2
S = 4096
NC = 8
T = S // 4            # tokens per core (1024)
NCH = T // 128        # 8 token chunks of 128
NBLK = T // BS        # 4 blocks per core
G = H // 2            # 16 head pairs (dim groups of 128)
KC = HID // 128       # 16 contraction chunks
LAYER_IDX = 0
NUM_LAYERS = 32
EPS = 1e-5

BF = ml_dtypes.bfloat16


def _decay():
    base = 1.0 / 2.0 ** (8.0 / H)
    rate = base ** (np.arange(H, dtype=np.float64) + 1.0)
    factor = 1.0 - LAYER_IDX / (NUM_LAYERS - 1 + 1e-5) + 1e-5
    slope = rate * factor                                  # (H,)
    r = np.arange(BS, dtype=np.float64) + 1.0
    qd = np.exp(-slope[:, None] * r[None, :])              # (H, BS) query decay
    kd = np.exp(-slope[:, None] * (BS - r[None, :]))       # (H, BS) key decay
    ij = r[:, None] - r[None, :]                           # i - j
    dd = np.where(
        ij[None] >= 0, np.exp(-slope[:, None, None] * ij[None]), 0.0
    )                                                      # (H, BS_i, BS_j)
    bd = np.exp(-slope * BS)                               # (H,) block decay
    return slope, qd, kd, dd, bd


def _build_nc():
    nc = bacc.Bacc(num_devices=NC)
    hsT = nc.declare_dram_parameter("hsT", [HID, T], BF16, isOutput=False)
    wvT = nc.declare_dram_parameter("wvT", [HID, H * D], BF16, isOutput=False)
    wqkT = nc.declare_dram_parameter("wqkT", [HID, G, 256], BF16, isOutput=False)
    gwT = nc.declare_dram_parameter("gwT", [HID, HID], BF16, isOutput=False)
    owT = nc.declare_dram_parameter("owT", [H * D, HID], BF16, isOutput=False)
    ddm = nc.declare_dram_parameter("ddm", [G, 128, 1024], BF16, isOutput=False)
    qdm = nc.declare_dram_parameter("qdm", [128, G, BS], BF16, isOutput=False)
    kdm = nc.declare_dram_parameter("kdm", [128, 2 * H], F32, isOutput=False)
    bdm = nc.declare_dram_parameter("bdm", [128, G], F32, isOutput=False)
    swm = nc.declare_dram_parameter("swm", [128, G * NC], F32, isOutput=False)
    nw = nc.declare_dram_parameter("nw", [128, 16], F32, isOutput=False)
    out = nc.declare_dram_parameter("out", [T, HID], F32, isOutput=True)

    eloc = nc.dram_tensor("eloc", [2, G, D, D], BF16)
    egath = nc.dram_tensor("egath", [NC, 2, G, D, D], BF16, addr_space="Shared")
    ssq_rt = nc.dram_tensor("ssq_rt", [T], F32)

    with tile.TileContext(nc, pool_alloc_mode="stack") as tc:
        # ---- constants -------------------------------------------------
        _c_ctx = ExitStack()
        c_pool = _c_ctx.enter_context(tc.tile_pool(name="c_pool", bufs=1))
        ident = c_pool.tile([128, 128], BF16, name="ident")
        make_identity(nc, ident[:])
        ones = c_pool.tile([128, 1], BF16, name="ones")
        nc.vector.memset(ones[:], 1.0)
        eps_sb = c_pool.tile([128, 1], F32, name="eps_sb")
        nc.vector.memset(eps_sb[:], EPS)
        kdm_sb = c_pool.tile([128, 2 * H], F32, name="kdm_sb")
        nc.sync.dma_start(kdm_sb[:], kdm[:])
        bdm_sb = c_pool.tile([128, G, 1], F32, name="bdm_sb")
        nc.sync.dma_start(bdm_sb[:, :, 0], bdm[:])
        # swm_sb[p, cc, g, 0] = sw[2g + p//64, cc]
        swm_sb = c_pool.tile([128, NC, G, 1], F32, name="swm_sb")
        nc.sync.dma_start(
            swm_sb[:, :, :, 0], swm.rearrange("p (c g) -> p c g", c=NC)
        )
        nw_sb = c_pool.tile([128, 16], F32, name="nw_sb")
        nc.sync.dma_start(nw_sb[:], nw[:])

        # ---- persistent activations ------------------------------------
        _xt_ctx = ExitStack()
        xt_pool = _xt_ctx.enter_context(tc.tile_pool(name="xt_pool", bufs=1))
        xT = xt_pool.tile([128, KC, T], BF16, name="xT")
        for q8 in range(8):
            nc.sync.dma_start(
                xT[:, q8 * 2 : (q8 + 1) * 2, :],
                hsT[q8 * 256 : (q8 + 1) * 256, :].rearrange(
                    "(k p) t -> p k t", p=128
                ),
            )
        _q_ctx = ExitStack()
        q_pool = _q_ctx.enter_context(tc.tile_pool(name="q_pool", bufs=1))
        qT_sb = q_pool.tile([128, G, T], BF16, name="qT_sb")
        _ys_ctx = ExitStack()
        ys_pool = _ys_ctx.enter_context(tc.tile_pool(name="ys_pool", bufs=1))
        ys_sb = ys_pool.tile([128, G, T], BF16, name="ys_sb")
        _ce_ctx = ExitStack()
        ce_pool = _ce_ctx.enter_context(tc.tile_pool(name="ce_pool", bufs=1))
        c_sb = ce_pool.tile([128, G, NBLK, D], BF16, name="c_sb")
        E_sb = ce_pool.tile([128, G, D], BF16, name="E_sb")
        # v_pool opened last among persistents: it is the only one released
        # mid-stream (stack allocator frees LIFO only)
        _v_ctx = ExitStack()
        v_pool = _v_ctx.enter_context(tc.tile_pool(name="v_pool", bufs=1))
        V_sb = v_pool.tile([128, NCH, H * D], BF16, name="V_sb")

        # ---- phase V: value projection (tok-major, all heads) -----------
        with tc.tile_pool(name="wv_p", bufs=3) as wv_p, tc.tile_pool(
            name="ps_v", bufs=1, space="PSUM"
        ) as ps_v:
            for n in range(4):
                pv = [
                    ps_v.tile([128, 512], F32, name=f"pv{m}") for m in range(NCH)
                ]
                for k in range(KC):
                    wv_t = wv_p.tile([128, 512], BF16, name="wv_t")
                    nc.sync.dma_start(
                        wv_t[:], wvT[k * 128 : (k + 1) * 128, n * 512 : (n + 1) * 512]
                    )
                    for m in range(NCH):
                        nc.tensor.matmul(
                            pv[m][:],
                            xT[:, k, m * 128 : (m + 1) * 128],
                            wv_t[:],
                            start=(k == 0),
                            stop=(k == KC - 1),
                        )
                for m in range(NCH):
                    nc.scalar.activation(
                        V_sb[:, m, n * 512 : (n + 1) * 512], pv[m][:], AF.Silu
                    )

        # ---- pass 1: per head-pair: QK proj, C/E summary, intra attn ----
        with tc.tile_pool(name="wqk_p", bufs=3) as wqk_p, tc.tile_pool(
            name="dd_p", bufs=2
        ) as dd_p, tc.tile_pool(name="kt_p", bufs=2) as kt_p, tc.tile_pool(
            name="ktok_p", bufs=2
        ) as ktok_p, tc.tile_pool(name="vkd_p", bufs=2) as vkd_p, tc.tile_pool(
            name="awm_p", bufs=3
        ) as awm_p, tc.tile_pool(name="ps1", bufs=1, space="PSUM") as ps1:
            for g in range(G):
                wqk_a = wqk_p.tile([128, 8, 256], BF16, name="wqk_t")
                nc.sync.dma_start(
                    wqk_a[:],
                    wqkT[0:1024, g, :].rearrange("(ko p) c -> p ko c", p=128),
                )
                wqk_b = wqk_p.tile([128, 8, 256], BF16, name="wqk_t")
                nc.sync.dma_start(
                    wqk_b[:],
                    wqkT[1024:2048, g, :].rearrange("(ko p) c -> p ko c", p=128),
                )
                dd_g = dd_p.tile([128, 1024], BF16, name="dd_g")
                nc.sync.dma_start(dd_g[:], ddm[g])

                # q projection (dim-major, both heads: partitions 0:64/64:128)
                for half in range(2):
                    pq = ps1.tile([128, 512], F32, name="pq", tag="proj", bufs=2)
                    for k in range(KC):
                        wt = wqk_a if k < 8 else wqk_b
                        nc.tensor.matmul(
                            pq[:],
                            wt[:, k % 8, 0:128],
                            xT[:, k, half * 512 : (half + 1) * 512],
                            start=(k == 0),
                            stop=(k == KC - 1),
                        )
                    nc.scalar.activation(
                        qT_sb[:, g, half * 512 : (half + 1) * 512], pq[:], AF.Silu
                    )
                # k projection
                kT2 = kt_p.tile([128, T], BF16, name="kT2")
                for half in range(2):
                    pk = ps1.tile([128, 512], F32, name="pk", tag="proj", bufs=2)
                    for k in range(KC):
                        wt = wqk_a if k < 8 else wqk_b
                        nc.tensor.matmul(
                            pk[:],
                            wt[:, k % 8, 128:256],
                            xT[:, k, half * 512 : (half + 1) * 512],
                            start=(k == 0),
                            stop=(k == KC - 1),
                        )
                    nc.scalar.activation(
                        kT2[:, half * 512 : (half + 1) * 512], pk[:], AF.Silu
                    )

                # k -> tok-major via PE transpose (8x 128x128, batched by 4)
                k_tok = ktok_p.tile([128, NCH, 128], BF16, name="k_tok")
                for b2 in range(2):
                    ptr = ps1.tile([128, 4, 128], BF16, name="ptr", tag="sm", bufs=3)
                    for ch in range(4):
                        cc = b2 * 4 + ch
                        nc.tensor.matmul(
                            ptr[:, ch, :],
                            kT2[:, cc * 128 : (cc + 1) * 128],
                            ident[:],
                            is_transpose=True,
                            skip_group_check=True,
                        )
                    nc.scalar.copy(
                        k_tok[:, b2 * 4 : (b2 + 1) * 4, :].rearrange(
                            "p a b -> p (a b)"
                        ),
                        ptr[:].rearrange("p a b -> p (a b)"),
                    )

                # v scaled by key-decay
                v_kd = vkd_p.tile([128, NCH, 128], BF16, name="v_kd")
                for m in range(NCH):
                    for hh in range(2):
                        h = 2 * g + hh
                        nc.vector.tensor_scalar_mul(
                            v_kd[:, m, hh * 64 : (hh + 1) * 64],
                            V_sb[:, m, g * 128 + hh * 64 : g * 128 + (hh + 1) * 64],
                            kdm_sb[:, 2 * h + (m % 2) : 2 * h + (m % 2) + 1],
                        )

                # block contributions C_jb = (k*kd)^T v (both heads at once)
                pc2 = ps1.tile([128, 4, 128], F32, name="pc2", tag="sm", bufs=3)
                for jb in range(NBLK):
                    for jc in range(2):
                        m = 2 * jb + jc
                        nc.tensor.matmul(
                            pc2[:, jb, :],
                            k_tok[:, m, :],
                            v_kd[:, m, :],
                            start=(jc == 0),
                            stop=(jc == 1),
                            skip_group_check=True,
                        )
                for hh in range(2):
                    sl = slice(hh * 64, (hh + 1) * 64)
                    nc.scalar.copy(
                        c_sb[sl, g, :, :], pc2[sl, :, hh * 64 : (hh + 1) * 64]
                    )
                # chunk summary E = sum_jb bd^(3-jb) C_jb
                nc.vector.tensor_copy(E_sb[:, g, :], c_sb[:, g, 0, :])
                for jb in range(1, NBLK):
                    nc.vector.scalar_tensor_tensor(
                        E_sb[:, g, :],
                        E_sb[:, g, :],
                        bdm_sb[:, g, :],
                        c_sb[:, g, jb, :],
                        ALU.mult,
                        ALU.add,
                    )

                # intra-block attention
                for jb in range(NBLK):
                    awms = []
                    for hh in range(2):
                        hs = slice(hh * 64, (hh + 1) * 64)
                        paw = ps1.tile([128, 2, 256], F32, name="paw", tag="aw", bufs=2)
                        for jc in range(2):
                            nc.tensor.matmul(
                                paw[:, jc, :],
                                kT2[hs, jb * 256 + jc * 128 : jb * 256 + (jc + 1) * 128],
                                qT_sb[hs, g, jb * 256 : (jb + 1) * 256],
                                start=True,
                                stop=True,
                                skip_group_check=True,
                            )
                        awm = awm_p.tile([128, 2, 256], BF16, name="awm")
                        nc.vector.tensor_mul(
                            awm[:].rearrange("p a b -> p (a b)"),
                            paw[:].rearrange("p a b -> p (a b)"),
                            dd_g[:, hh * 512 : (hh + 1) * 512],
                        )
                        awms.append(awm)
                    pys = ps1.tile([128, 256], F32, name="pys", tag="sm", bufs=3)
                    for hh in range(2):
                        for jc in range(2):
                            m = 2 * jb + jc
                            nc.tensor.matmul(
                                pys[hh * 64 : (hh + 1) * 64, :],
                                V_sb[:, m, g * 128 + hh * 64 : g * 128 + (hh + 1) * 64],
                                awms[hh][:, jc, :],
                                start=(jc == 0),
                                stop=(jc == 1),
                                skip_group_check=True,
                            )
                    nc.scalar.copy(ys_sb[:, g, jb * 256 : (jb + 1) * 256], pys[:])

        # ---- collective: share per-chunk KV summaries -------------------
        nc.sync.dma_start(eloc[0].rearrange("g d e -> d g e"), E_sb[0:64, :, :])
        nc.sync.dma_start(eloc[1].rearrange("g d e -> d g e"), E_sb[64:128, :, :])
        nc.gpsimd.collective_compute(
            "AllGather",
            ALU.bypass,
            replica_groups=[list(range(NC))],
            ins=[eloc[:]],
            outs=[egath[:]],
        )
        _v_ctx.close()

        # ---- gate projection (overlaps the collective) ------------------
        _g_ctx = ExitStack()
        g_pool = _g_ctx.enter_context(tc.tile_pool(name="g_pool", bufs=1))
        gate_sb = g_pool.tile([128, G, T], BF16, name="gate_sb")
        _eg_ctx = ExitStack()
        eg_pool = _eg_ctx.enter_context(tc.tile_pool(name="eg_pool", bufs=1))
        EG = eg_pool.tile([128, G, NC, D], BF16, name="EG")
        qd_sb = eg_pool.tile([128, G, BS], BF16, name="qd_sb")
        nc.sync.dma_start(qd_sb[:], qdm[:])
        for cc in range(NC):
            nc.sync.dma_start(
                EG[0:64, :, cc, :], egath[cc, 0].rearrange("g d e -> d g e")
            )
            nc.sync.dma_start(
                EG[64:128, :, cc, :], egath[cc, 1].rearrange("g d e -> d g e")
            )

        # ss4[:, jb, g, :] = chunk-start state for local block jb of pair g
        ss4 = g_pool.tile([128, NBLK, G, D], BF16, name="ss4")

        # ---- merged phase: gate proj + pass 2 (inter) + rmsnorm prep ----
        # pass-2 / prep work for group i-LAG is interleaved after gate
        # chunk i so the collective latency hides under the first LAG
        # gate chunks and the vector work overlaps gate matmuls.
        LAG = 6
        with tc.tile_pool(name="gw_p", bufs=3) as gw_p, tc.tile_pool(
            name="tmp_p", bufs=2
        ) as tmp_p, tc.tile_pool(name="sq_p2", bufs=2) as sq_p2, tc.tile_pool(
            name="ps_g", bufs=1, space="PSUM"
        ) as ps_g, tc.tile_pool(name="ps2", bufs=1, space="PSUM") as ps2, tc.tile_pool(
            name="ps_sq", bufs=1, space="PSUM"
        ) as ps_sq:
            # batched chunk-start state computation (vector; waits on EG)
            nc.vector.tensor_mul(
                ss4[:, 0, :, :],
                EG[:, :, 0, :],
                swm_sb[:, 0, :, :].broadcast_to([128, G, D]),
            )
            for cc in range(1, NC):
                tmp = tmp_p.tile([128, G, D], BF16, name="tmp")
                nc.vector.tensor_mul(
                    tmp[:],
                    EG[:, :, cc, :],
                    swm_sb[:, cc, :, :].broadcast_to([128, G, D]),
                )
                nc.vector.tensor_add(ss4[:, 0, :, :], ss4[:, 0, :, :], tmp[:])
            for jb in range(1, NBLK):
                tmp = tmp_p.tile([128, G, D], BF16, name="tmp")
                nc.vector.tensor_mul(
                    tmp[:],
                    ss4[:, jb - 1, :, :],
                    bdm_sb[:, :, :].broadcast_to([128, G, D]),
                )
                nc.vector.tensor_add(
                    ss4[:, jb, :, :], tmp[:], c_sb[:, :, jb - 1, :]
                )

            ssq0 = ps_sq.tile([1, 512], F32, name="ssq0")
            ssq1 = ps_sq.tile([1, 512], F32, name="ssq1")

            def pass2_group(g):
                pin = ps2.tile([128, NBLK, BS], F32, name="pin", bufs=2)
                for jb in range(NBLK):
                    for hh in range(2):
                        hs = slice(hh * 64, (hh + 1) * 64)
                        nc.tensor.matmul(
                            pin[hs, jb, :],
                            ss4[hs, jb, g, :],
                            qT_sb[hs, g, jb * 256 : (jb + 1) * 256],
                            start=True,
                            stop=True,
                            skip_group_check=True,
                        )
                tmp3 = tmp_p.tile([128, NBLK, BS], BF16, name="tmp3")
                nc.vector.tensor_mul(
                    tmp3[:],
                    pin[:],
                    qd_sb[:, g : g + 1, :].broadcast_to([128, NBLK, BS]),
                )
                nc.vector.tensor_add(
                    ys_sb[:, g, :],
                    ys_sb[:, g, :],
                    tmp3[:].rearrange("p a b -> p (a b)"),
                )
                # rmsnorm prep for this (now final) chunk of ys
                sq = sq_p2.tile([128, T], BF16, name="sq")
                nc.scalar.activation(sq[:], ys_sb[:, g, :], AF.Square)
                for half in range(2):
                    nc.tensor.matmul(
                        [ssq0, ssq1][half][:],
                        ones[:],
                        sq[:, half * 512 : (half + 1) * 512],
                        start=(g == 0),
                        stop=(g == G - 1),
                        skip_group_check=True,
                    )
                nc.vector.scalar_tensor_tensor(
                    gate_sb[:, g, :],
                    ys_sb[:, g, :],
                    nw_sb[:, g : g + 1],
                    gate_sb[:, g, :],
                    ALU.mult,
                    ALU.mult,
                )  # gate <- ys * norm_w * gate (per dim-chunk g)

            for i in range(16 + LAG):
                if i < 16:
                    gm = i
                    gw_t = gw_p.tile([128, KC, 128], BF16, name="gw_t")
                    nc.sync.dma_start(
                        gw_t[:],
                        gwT[:, gm * 128 : (gm + 1) * 128].rearrange(
                            "(ko p) c -> p ko c", p=128
                        ),
                    )
                    for gn in range(2):
                        pg = ps_g.tile([128, 512], F32, name="pg", bufs=2)
                        for k in range(KC):
                            nc.tensor.matmul(
                                pg[:],
                                gw_t[:, k, :],
                                xT[:, k, gn * 512 : (gn + 1) * 512],
                                start=(k == 0),
                                stop=(k == KC - 1),
                            )
                        nc.scalar.activation(
                            gate_sb[:, gm, gn * 512 : (gn + 1) * 512],
                            pg[:],
                            AF.Sigmoid,
                        )
                if i >= LAG:
                    pass2_group(i - LAG)

            ssq_sb = sq_p2.tile([1, T], F32, name="ssq_sb")
            nc.vector.tensor_copy(ssq_sb[:, 0:512], ssq0[:])
            nc.vector.tensor_copy(ssq_sb[:, 512:1024], ssq1[:])
            nc.sync.dma_start(ssq_rt[:], ssq_sb[:])

        # ---- phase F: output projection --------------------------------
        with tc.tile_pool(name="sq_p", bufs=2) as sq_p:
            ns_l = sq_p.tile([128, NCH], F32, name="ns_l")
            nc.sync.dma_start(ns_l[:], ssq_rt.rearrange("(c p) -> p c", p=128))
            ns_t = sq_p.tile([128, NCH], F32, name="ns_t")
            nc.scalar.activation(
                ns_t[:], ns_l[:], AF.Sqrt, bias=eps_sb[:, 0:1], scale=1.0 / (H * D)
            )
            ns_sb = sq_p.tile([128, NCH], F32, name="ns_sb")
            nc.vector.reciprocal(ns_sb[:], ns_t[:])

            with tc.tile_pool(name="ow_p", bufs=4) as ow_p, tc.tile_pool(
                name="oo_p", bufs=3
            ) as oo_p, tc.tile_pool(name="ps_o", bufs=1, space="PSUM") as ps_o:
                for n in range(4):
                    po = [
                        ps_o.tile([128, 512], F32, name=f"po{m}") for m in range(NCH)
                    ]
                    for k in range(KC):
                        ow_t = ow_p.tile([128, 512], BF16, name="ow_t")
                        nc.sync.dma_start(
                            ow_t[:],
                            owT[k * 128 : (k + 1) * 128, n * 512 : (n + 1) * 512],
                        )
                        for m in range(NCH):
                            nc.tensor.matmul(
                                po[m][:],
                                gate_sb[:, k, m * 128 : (m + 1) * 128],
                                ow_t[:],
                                start=(k == 0),
                                stop=(k == KC - 1),
                            )
                    for m in range(NCH):
                        oo_t = oo_p.tile([128, 512], F32, name="oo_t")
                        nc.scalar.mul(oo_t[:], po[m][:], ns_sb[:, m : m + 1])
                        nc.sync.dma_start(
                            out[m * 128 : (m + 1) * 128, n * 512 : (n + 1) * 512],
                            oo_t[:],
                        )
        _eg_ctx.close()
        _g_ctx.close()
        _ce_ctx.close()
        _ys_ctx.close()
        _q_ctx.close()
        _xt_ctx.close()
        _c_ctx.close()
    nc.finalize()
    return nc


_CACHE = {}


def _get_nc():
    if "nc" not in _CACHE:
        _CACHE["nc"] = _build_nc()
    return _CACHE["nc"]


def _host_prep(hidden_states, qkv_w, out_w, gate_w, norm_w):
    slope, qd, kd, dd, bd = _decay()
    w3 = qkv_w.reshape(H, 3 * D, HID)
    wq = w3[:, 0:D, :].reshape(H * D, HID)
    wk = w3[:, D : 2 * D, :].reshape(H * D, HID)
    wv = w3[:, 2 * D : 3 * D, :].reshape(H * D, HID)
    # wqkT[:, g, 0:128] = q dims of heads 2g,2g+1; [:, g, 128:256] = k dims
    wqkT = np.concatenate(
        [
            np.ascontiguousarray(wq.T).reshape(HID, G, 128),
            np.ascontiguousarray(wk.T).reshape(HID, G, 128),
        ],
        axis=2,
    ).astype(BF)
    wvT = np.ascontiguousarray(wv.T).astype(BF)
    gwT = np.ascontiguousarray(gate_w.T).astype(BF)
    owT = np.ascontiguousarray(out_w.T).astype(BF)

    # ddm[g, p, h*512 + jc*256 + i] = dd[2g+h, i, jc*128+p]
    dd_t = dd.transpose(0, 2, 1)  # (H, j, i)
    ddm = np.ascontiguousarray(
        dd_t.reshape(G, 2, 2, 128, BS).transpose(0, 3, 1, 2, 4).reshape(G, 128, 1024)
    ).astype(BF)
    # qdm[p, g, i] = qd[2g + p//64, i]
    qdm = np.ascontiguousarray(
        np.broadcast_to(qd.reshape(G, 2, 1, BS), (G, 2, 64, BS))
        .transpose(1, 2, 0, 3)
        .reshape(128, G, BS)
    ).astype(BF)
    # kdm[p, 2h+parity] = kd[h, parity*128+p]
    kdm = np.ascontiguousarray(
        kd.reshape(H, 2, 128).transpose(2, 0, 1).reshape(128, 2 * H)
    ).astype(np.float32)
    # bdm[p, g] = bd[2g + p//64]
    bdm = np.ascontiguousarray(
        np.broadcast_to(bd.reshape(G, 2, 1), (G, 2, 64)).transpose(1, 2, 0).reshape(128, G)
    ).astype(np.float32)
    nw = np.ascontiguousarray(norm_w.reshape(16, 128).T).astype(np.float32)

    shared = dict(wqkT=wqkT, wvT=wvT, gwT=gwT, owT=owT, ddm=ddm, qdm=qdm,
                  kdm=kdm, bdm=bdm, nw=nw)
    in_maps = []
    for c in range(NC):
        bb, p = c // 4, c % 4
        hsT = np.ascontiguousarray(
            hidden_states[bb, p * T : (p + 1) * T, :].T
        ).astype(BF)
        sw = np.zeros((H, NC), dtype=np.float64)
        for cc in range(NC):
            if cc // 4 == bb and cc % 4 < p:
                sw[:, cc] = bd ** (4.0 * (p - 1 - (cc % 4)))
        # swm[p_, cc*G+g] = sw[2g + p_//64, cc]  (cc-major)
        swm = np.ascontiguousarray(
            np.broadcast_to(sw.reshape(G, 2, 1, NC), (G, 2, 64, NC))
            .transpose(1, 2, 3, 0)
            .reshape(128, NC * G)
        ).astype(np.float32)
        in_maps.append(dict(hsT=hsT, swm=swm, **shared))
    return in_maps


def _run(inputs, trace=False):
    nc = _get_nc()
    in_maps = _host_prep(
        np.asarray(inputs["hidden_states"], dtype=np.float32),
        np.asarray(inputs["qkv_w"], dtype=np.float32),
        np.asarray(inputs["out_w"], dtype=np.float32),
        np.asarray(inputs["gate_w"], dtype=np.float32),
        np.asarray(inputs["norm_w"], dtype=np.float32),
    )
    res = run_bass_kernel_spmd(nc, in_maps, core_ids=list(range(NC)), trace=trace)
    full = np.empty((B, S, HID), dtype=np.float32)
    for c in range(NC):
        bb, p = c // 4, c % 4
        full[bb, p * T : (p + 1) * T, :] = res.results[c]["out"]
    return full, res


def kernel(**inputs):
    return _run(inputs, trace=False)[0]


def kernel_traced(**inputs):
    full, res = _run(inputs, trace=True)
    return full, res.exec_time_ns


# revision 28
# speedup vs baseline: 1.5999x; 1.0718x over previous
"""MiniMax Lightning Attention on 8 Trainium2 NeuronCores.

Sharding: sequence-parallel. Core c handles batch c//4, token chunk
(c%4)*1024..+1024 (4 blocks of 256). The per-block decay-state recurrence
crosses chunk boundaries; each core computes its local per-chunk decay-
weighted KV summary E, an AllGather shares the 8 summaries, and each core
reconstructs its chunk-start state as a decay-weighted sum.

Fused single-residency design: all big activations (x, V, q, ys, gate)
stay SBUF-resident in bf16; weights stream in bf16. Heads are processed
in pairs (dim groups of 128) so projections, transposes, and PSUM tiles
use full 128-partition width. Intra-block attention runs in pass 1
(before the collective); the cross-chunk "inter" term is a cheap pass 2
after the AllGather, hidden behind the gate projection.
"""

import numpy as np
import ml_dtypes

from contextlib import ExitStack

import concourse.bacc as bacc
import concourse.mybir as mybir
import concourse.tile as tile
from concourse.bass_utils import run_bass_kernel_spmd
from concourse.masks import make_identity

AF = mybir.ActivationFunctionType
ALU = mybir.AluOpType
F32 = mybir.dt.float32
F32R = mybir.dt.float32r
BF16 = mybir.dt.bfloat16

H = 32
D = 64
BS = 256
HID = 2048
B = 2
S = 4096
NC = 8
T = S // 4            # tokens per core (1024)
NCH = T // 128        # 8 token chunks of 128
NBLK = T // BS        # 4 blocks per core
G = H // 2            # 16 head pairs (dim groups of 128)
KC = HID // 128       # 16 contraction chunks
LAYER_IDX = 0
NUM_LAYERS = 32
EPS = 1e-5

BF = ml_dtypes.bfloat16


def _decay():
    base = 1.0 / 2.0 ** (8.0 / H)
    rate = base ** (np.arange(H, dtype=np.float64) + 1.0)
    factor = 1.0 - LAYER_IDX / (NUM_LAYERS - 1 + 1e-5) + 1e-5
    slope = rate * factor                                  # (H,)
    r = np.arange(BS, dtype=np.float64) + 1.0
    qd = np.exp(-slope[:, None] * r[None, :])              # (H, BS) query decay
    kd = np.exp(-slope[:, None] * (BS - r[None, :]))       # (H, BS) key decay
    ij = r[:, None] - r[None, :]                           # i - j
    dd = np.where(
        ij[None] >= 0, np.exp(-slope[:, None, None] * ij[None]), 0.0
    )                                                      # (H, BS_i, BS_j)
    bd = np.exp(-slope * BS)                               # (H,) block decay
    return slope, qd, kd, dd, bd


def _build_nc():
    nc = bacc.Bacc(num_devices=NC)
    hsT = nc.declare_dram_parameter("hsT", [HID, T], BF16, isOutput=False)
    wvT = nc.declare_dram_parameter("wvT", [HID, H * D], BF16, isOutput=False)
    wqkT = nc.declare_dram_parameter("wqkT", [HID, G, 256], BF16, isOutput=False)
    gwT = nc.declare_dram_parameter("gwT", [HID, HID], BF16, isOutput=False)
    owT = nc.declare_dram_parameter("owT", [H * D, HID], BF16, isOutput=False)
    ddm = nc.declare_dram_parameter("ddm", [G, 128, 1024], BF16, isOutput=False)
    qdm = nc.declare_dram_parameter("qdm", [128, G, BS], BF16, isOutput=False)
    kdm = nc.declare_dram_parameter("kdm", [128, 2 * H], F32, isOutput=False)
    bdm = nc.declare_dram_parameter("bdm", [128, G], F32, isOutput=False)
    swm = nc.declare_dram_parameter("swm", [128, G * NC], F32, isOutput=False)
    nw = nc.declare_dram_parameter("nw", [128, 16], F32, isOutput=False)
    out = nc.declare_dram_parameter("out", [T, HID], F32, isOutput=True)

    # collective split in two group-halves so the first AllGather hides
    # under the second half of pass 1
    eloc_a = nc.dram_tensor("eloc_a", [2, G // 2, D, D], BF16)
    eloc_b = nc.dram_tensor("eloc_b", [2, G // 2, D, D], BF16)
    egath_a = nc.dram_tensor("egath_a", [NC, 2, G // 2, D, D], BF16, addr_space="Shared")
    egath_b = nc.dram_tensor("egath_b", [NC, 2, G // 2, D, D], BF16, addr_space="Shared")
    ssq_rt = nc.dram_tensor("ssq_rt", [T], F32)

    with tile.TileContext(nc, pool_alloc_mode="stack") as tc:
        # ---- constants -------------------------------------------------
        _c_ctx = ExitStack()
        c_pool = _c_ctx.enter_context(tc.tile_pool(name="c_pool", bufs=1))
        ident = c_pool.tile([128, 128], BF16, name="ident")
        make_identity(nc, ident[:])
        ones = c_pool.tile([128, 1], BF16, name="ones")
        nc.vector.memset(ones[:], 1.0)
        eps_sb = c_pool.tile([128, 1], F32, name="eps_sb")
        nc.vector.memset(eps_sb[:], EPS)
        kdm_sb = c_pool.tile([128, 2 * H], F32, name="kdm_sb")
        nc.sync.dma_start(kdm_sb[:], kdm[:])
        bdm_sb = c_pool.tile([128, G, 1], F32, name="bdm_sb")
        nc.sync.dma_start(bdm_sb[:, :, 0], bdm[:])
        # swm_sb[p, cc, g, 0] = sw[2g + p//64, cc]
        swm_sb = c_pool.tile([128, NC, G, 1], F32, name="swm_sb")
        nc.sync.dma_start(
            swm_sb[:, :, :, 0], swm.rearrange("p (c g) -> p c g", c=NC)
        )
        nw_sb = c_pool.tile([128, 16], F32, name="nw_sb")
        nc.sync.dma_start(nw_sb[:], nw[:])

        # ---- persistent activations ------------------------------------
        _xt_ctx = ExitStack()
        xt_pool = _xt_ctx.enter_context(tc.tile_pool(name="xt_pool", bufs=1))
        xT = xt_pool.tile([128, KC, T], BF16, name="xT")
        for q8 in range(8):
            nc.sync.dma_start(
                xT[:, q8 * 2 : (q8 + 1) * 2, :],
                hsT[q8 * 256 : (q8 + 1) * 256, :].rearrange(
                    "(k p) t -> p k t", p=128
                ),
            )
        _q_ctx = ExitStack()
        q_pool = _q_ctx.enter_context(tc.tile_pool(name="q_pool", bufs=1))
        qT_sb = q_pool.tile([128, G, T], BF16, name="qT_sb")
        _ys_ctx = ExitStack()
        ys_pool = _ys_ctx.enter_context(tc.tile_pool(name="ys_pool", bufs=1))
        ys_sb = ys_pool.tile([128, G, T], BF16, name="ys_sb")
        _ce_ctx = ExitStack()
        ce_pool = _ce_ctx.enter_context(tc.tile_pool(name="ce_pool", bufs=1))
        c_sb = ce_pool.tile([128, G, NBLK, D], BF16, name="c_sb")
        E_sb = ce_pool.tile([128, G, D], BF16, name="E_sb")
        # v_pool opened last among persistents: it is the only one released
        # mid-stream (stack allocator frees LIFO only)
        _v_ctx = ExitStack()
        v_pool = _v_ctx.enter_context(tc.tile_pool(name="v_pool", bufs=1))
        V_sb = v_pool.tile([128, NCH, H * D], BF16, name="V_sb")

        # ---- phase V: value projection (tok-major, all heads) -----------
        with tc.tile_pool(name="wv_p", bufs=4) as wv_p, tc.tile_pool(
            name="ps_v", bufs=1, space="PSUM"
        ) as ps_v:
            for n in range(4):
                pv = [
                    ps_v.tile([128, 512], F32, name=f"pv{m}") for m in range(NCH)
                ]
                for k in range(KC):
                    wv_t = wv_p.tile([128, 512], BF16, name="wv_t")
                    nc.sync.dma_start(
                        wv_t[:], wvT[k * 128 : (k + 1) * 128, n * 512 : (n + 1) * 512]
                    )
                    for m in range(NCH):
                        nc.tensor.matmul(
                            pv[m][:],
                            xT[:, k, m * 128 : (m + 1) * 128],
                            wv_t[:],
                            start=(k == 0),
                            stop=(k == KC - 1),
                        )
                for m in range(NCH):
                    nc.scalar.activation(
                        V_sb[:, m, n * 512 : (n + 1) * 512], pv[m][:], AF.Silu
                    )

        # ---- pass 1: per head-pair: QK proj, C/E summary, intra attn ----
        with tc.tile_pool(name="wqk_p", bufs=3) as wqk_p, tc.tile_pool(
            name="dd_p", bufs=2
        ) as dd_p, tc.tile_pool(name="kt_p", bufs=2) as kt_p, tc.tile_pool(
            name="ktok_p", bufs=2
        ) as ktok_p, tc.tile_pool(name="vkd_p", bufs=2) as vkd_p, tc.tile_pool(
            name="awm_p", bufs=3
        ) as awm_p, tc.tile_pool(name="ps1", bufs=1, space="PSUM") as ps1:
            for g in range(G):
                wqk_a = wqk_p.tile([128, 8, 256], BF16, name="wqk_t")
                nc.sync.dma_start(
                    wqk_a[:],
                    wqkT[0:1024, g, :].rearrange("(ko p) c -> p ko c", p=128),
                )
                wqk_b = wqk_p.tile([128, 8, 256], BF16, name="wqk_t")
                nc.sync.dma_start(
                    wqk_b[:],
                    wqkT[1024:2048, g, :].rearrange("(ko p) c -> p ko c", p=128),
                )
                dd_g = dd_p.tile([128, 1024], BF16, name="dd_g")
                nc.sync.dma_start(dd_g[:], ddm[g])

                # q projection (dim-major, both heads: partitions 0:64/64:128)
                for half in range(2):
                    pq = ps1.tile([128, 512], F32, name="pq", tag="proj", bufs=2)
                    for k in range(KC):
                        wt = wqk_a if k < 8 else wqk_b
                        nc.tensor.matmul(
                            pq[:],
                            wt[:, k % 8, 0:128],
                            xT[:, k, half * 512 : (half + 1) * 512],
                            start=(k == 0),
                            stop=(k == KC - 1),
                        )
                    nc.scalar.activation(
                        qT_sb[:, g, half * 512 : (half + 1) * 512], pq[:], AF.Silu
                    )
                # k projection
                kT2 = kt_p.tile([128, T], BF16, name="kT2")
                for half in range(2):
                    pk = ps1.tile([128, 512], F32, name="pk", tag="proj", bufs=2)
                    for k in range(KC):
                        wt = wqk_a if k < 8 else wqk_b
                        nc.tensor.matmul(
                            pk[:],
                            wt[:, k % 8, 128:256],
                            xT[:, k, half * 512 : (half + 1) * 512],
                            start=(k == 0),
                            stop=(k == KC - 1),
                        )
                    nc.scalar.activation(
                        kT2[:, half * 512 : (half + 1) * 512], pk[:], AF.Silu
                    )

                # k -> tok-major via PE transpose (8x 128x128, batched by 4)
                k_tok = ktok_p.tile([128, NCH, 128], BF16, name="k_tok")
                for b2 in range(2):
                    ptr = ps1.tile([128, 4, 128], BF16, name="ptr", tag="sm", bufs=3)
                    for ch in range(4):
                        cc = b2 * 4 + ch
                        nc.tensor.matmul(
                            ptr[:, ch, :],
                            kT2[:, cc * 128 : (cc + 1) * 128],
                            ident[:],
                            is_transpose=True,
                            skip_group_check=True,
                        )
                    nc.scalar.copy(
                        k_tok[:, b2 * 4 : (b2 + 1) * 4, :].rearrange(
                            "p a b -> p (a b)"
                        ),
                        ptr[:].rearrange("p a b -> p (a b)"),
                    )

                # v scaled by key-decay
                v_kd = vkd_p.tile([128, NCH, 128], BF16, name="v_kd")
                for m in range(NCH):
                    for hh in range(2):
                        h = 2 * g + hh
                        nc.vector.tensor_scalar_mul(
                            v_kd[:, m, hh * 64 : (hh + 1) * 64],
                            V_sb[:, m, g * 128 + hh * 64 : g * 128 + (hh + 1) * 64],
                            kdm_sb[:, 2 * h + (m % 2) : 2 * h + (m % 2) + 1],
                        )

                # block contributions C_jb = (k*kd)^T v (both heads at once)
                pc2 = ps1.tile([128, 4, 128], F32, name="pc2", tag="sm", bufs=3)
                for jb in range(NBLK):
                    for jc in range(2):
                        m = 2 * jb + jc
                        nc.tensor.matmul(
                            pc2[:, jb, :],
                            k_tok[:, m, :],
                            v_kd[:, m, :],
                            start=(jc == 0),
                            stop=(jc == 1),
                            skip_group_check=True,
                        )
                for hh in range(2):
                    sl = slice(hh * 64, (hh + 1) * 64)
                    nc.scalar.copy(
                        c_sb[sl, g, :, :], pc2[sl, :, hh * 64 : (hh + 1) * 64]
                    )
                # chunk summary E = sum_jb bd^(3-jb) C_jb
                nc.vector.tensor_copy(E_sb[:, g, :], c_sb[:, g, 0, :])
                for jb in range(1, NBLK):
                    nc.vector.scalar_tensor_tensor(
                        E_sb[:, g, :],
                        E_sb[:, g, :],
                        bdm_sb[:, g, :],
                        c_sb[:, g, jb, :],
                        ALU.mult,
                        ALU.add,
                    )

                # intra-block attention
                for jb in range(NBLK):
                    awms = []
                    for hh in range(2):
                        hs = slice(hh * 64, (hh + 1) * 64)
                        paw = ps1.tile([128, 2, 256], F32, name="paw", tag="aw", bufs=2)
                        for jc in range(2):
                            nc.tensor.matmul(
                                paw[:, jc, :],
                                kT2[hs, jb * 256 + jc * 128 : jb * 256 + (jc + 1) * 128],
                                qT_sb[hs, g, jb * 256 : (jb + 1) * 256],
                                start=True,
                                stop=True,
                                skip_group_check=True,
                            )
                        awm = awm_p.tile([128, 2, 256], BF16, name="awm")
                        nc.vector.tensor_mul(
                            awm[:].rearrange("p a b -> p (a b)"),
                            paw[:].rearrange("p a b -> p (a b)"),
                            dd_g[:, hh * 512 : (hh + 1) * 512],
                        )
                        awms.append(awm)
                    pys = ps1.tile([128, 256], F32, name="pys", tag="sm", bufs=3)
                    for hh in range(2):
                        for jc in range(2):
                            m = 2 * jb + jc
                            nc.tensor.matmul(
                                pys[hh * 64 : (hh + 1) * 64, :],
                                V_sb[:, m, g * 128 + hh * 64 : g * 128 + (hh + 1) * 64],
                                awms[hh][:, jc, :],
                                start=(jc == 0),
                                stop=(jc == 1),
                                skip_group_check=True,
                            )
                    nc.scalar.copy(ys_sb[:, g, jb * 256 : (jb + 1) * 256], pys[:])

                # first-half collective fires mid-pass-1 so its latency
                # hides under the remaining groups' compute
                if g == G // 2 - 1 or g == G - 1:
                    eh, lo = (eloc_a, 0) if g < G // 2 else (eloc_b, G // 2)
                    nc.sync.dma_start(
                        eh[0].rearrange("g d e -> d g e"),
                        E_sb[0:64, lo : lo + G // 2, :],
                    )
                    nc.sync.dma_start(
                        eh[1].rearrange("g d e -> d g e"),
                        E_sb[64:128, lo : lo + G // 2, :],
                    )
                    gh = egath_a if g < G // 2 else egath_b
                    nc.gpsimd.collective_compute(
                        "AllGather",
                        ALU.bypass,
                        replica_groups=[list(range(NC))],
                        ins=[eh[:]],
                        outs=[gh[:]],
                    )
        _v_ctx.close()

        # ---- gate projection (overlaps the collective) ------------------
        _g_ctx = ExitStack()
        g_pool = _g_ctx.enter_context(tc.tile_pool(name="g_pool", bufs=1))
        gate_sb = g_pool.tile([128, G, T], BF16, name="gate_sb")
        _eg_ctx = ExitStack()
        eg_pool = _eg_ctx.enter_context(tc.tile_pool(name="eg_pool", bufs=1))
        EG = eg_pool.tile([128, G, NC, D], BF16, name="EG")
        qd_sb = eg_pool.tile([128, G, BS], BF16, name="qd_sb")
        nc.sync.dma_start(qd_sb[:], qdm[:])
        for hf, gh in ((0, egath_a), (1, egath_b)):
            lo = hf * (G // 2)
            for cc in range(NC):
                nc.sync.dma_start(
                    EG[0:64, lo : lo + G // 2, cc, :],
                    gh[cc, 0].rearrange("g d e -> d g e"),
                )
                nc.sync.dma_start(
                    EG[64:128, lo : lo + G // 2, cc, :],
                    gh[cc, 1].rearrange("g d e -> d g e"),
                )

        # ss4[:, jb, g, :] = chunk-start state for local block jb of pair g
        ss4 = g_pool.tile([128, NBLK, G, D], BF16, name="ss4")

        # ---- merged phase: gate proj + pass 2 (inter) + rmsnorm prep ----
        # pass-2 / prep work for group i-LAG is interleaved after gate
        # chunk i so the collective latency hides under the first LAG
        # gate chunks and the vector work overlaps gate matmuls.
        LAG = 2
        GH = G // 2
        with tc.tile_pool(name="gw_p", bufs=3) as gw_p, tc.tile_pool(
            name="tmp_p", bufs=2
        ) as tmp_p, tc.tile_pool(name="sq_p2", bufs=2) as sq_p2, tc.tile_pool(
            name="ps_g", bufs=1, space="PSUM"
        ) as ps_g, tc.tile_pool(name="ps2", bufs=1, space="PSUM") as ps2, tc.tile_pool(
            name="ps_sq", bufs=1, space="PSUM"
        ) as ps_sq:
            # batched chunk-start state computation per group-half
            # (vector; each half waits only on its own AllGather)
            def ss4_init(hf):
                gs = slice(hf * GH, (hf + 1) * GH)
                nc.vector.tensor_mul(
                    ss4[:, 0, gs, :],
                    EG[:, gs, 0, :],
                    swm_sb[:, 0, gs, :].broadcast_to([128, GH, D]),
                )
                for cc in range(1, NC):
                    tmp = tmp_p.tile([128, GH, D], BF16, name="tmp")
                    nc.vector.tensor_mul(
                        tmp[:],
                        EG[:, gs, cc, :],
                        swm_sb[:, cc, gs, :].broadcast_to([128, GH, D]),
                    )
                    nc.vector.tensor_add(ss4[:, 0, gs, :], ss4[:, 0, gs, :], tmp[:])
                for jb in range(1, NBLK):
                    tmp = tmp_p.tile([128, GH, D], BF16, name="tmp")
                    nc.vector.tensor_mul(
                        tmp[:],
                        ss4[:, jb - 1, gs, :],
                        bdm_sb[:, gs, :].broadcast_to([128, GH, D]),
                    )
                    nc.vector.tensor_add(
                        ss4[:, jb, gs, :], tmp[:], c_sb[:, gs, jb - 1, :]
                    )

            ss4_init(0)
            ssq0 = ps_sq.tile([1, 512], F32, name="ssq0")
            ssq1 = ps_sq.tile([1, 512], F32, name="ssq1")

            def pass2_group(g):
                pin = ps2.tile([128, NBLK, BS], F32, name="pin", bufs=2)
                for jb in range(NBLK):
                    for hh in range(2):
                        hs = slice(hh * 64, (hh + 1) * 64)
                        nc.tensor.matmul(
                            pin[hs, jb, :],
                            ss4[hs, jb, g, :],
                            qT_sb[hs, g, jb * 256 : (jb + 1) * 256],
                            start=True,
                            stop=True,
                            skip_group_check=True,
                        )
                tmp3 = tmp_p.tile([128, NBLK, BS], BF16, name="tmp3")
                nc.vector.tensor_mul(
                    tmp3[:],
                    pin[:],
                    qd_sb[:, g : g + 1, :].broadcast_to([128, NBLK, BS]),
                )
                nc.vector.tensor_add(
                    ys_sb[:, g, :],
                    ys_sb[:, g, :],
                    tmp3[:].rearrange("p a b -> p (a b)"),
                )
                # rmsnorm prep for this (now final) chunk of ys
                sq = sq_p2.tile([128, T], BF16, name="sq")
                nc.scalar.activation(sq[:], ys_sb[:, g, :], AF.Square)
                for half in range(2):
                    nc.tensor.matmul(
                        [ssq0, ssq1][half][:],
                        ones[:],
                        sq[:, half * 512 : (half + 1) * 512],
                        start=(g == 0),
                        stop=(g == G - 1),
                        skip_group_check=True,
                    )
                nc.vector.scalar_tensor_tensor(
                    gate_sb[:, g, :],
                    ys_sb[:, g, :],
                    nw_sb[:, g : g + 1],
                    gate_sb[:, g, :],
                    ALU.mult,
                    ALU.mult,
                )  # gate <- ys * norm_w * gate (per dim-chunk g)

            for i in range(16 + LAG):
                if i == 4:
                    ss4_init(1)
                if i < 16:
                    gm = i
                    gw_t = gw_p.tile([128, KC, 128], BF16, name="gw_t")
                    nc.sync.dma_start(
                        gw_t[:],
                        gwT[:, gm * 128 : (gm + 1) * 128].rearrange(
                            "(ko p) c -> p ko c", p=128
                        ),
                    )
                    for gn in range(2):
                        pg = ps_g.tile([128, 512], F32, name="pg", bufs=2)
                        for k in range(KC):
                            nc.tensor.matmul(
                                pg[:],
                                gw_t[:, k, :],
                                xT[:, k, gn * 512 : (gn + 1) * 512],
                                start=(k == 0),
                                stop=(k == KC - 1),
                            )
                        nc.scalar.activation(
                            gate_sb[:, gm, gn * 512 : (gn + 1) * 512],
                            pg[:],
                            AF.Sigmoid,
                        )
                if i >= LAG:
                    pass2_group(i - LAG)

            ssq_sb = sq_p2.tile([1, T], F32, name="ssq_sb")
            nc.vector.tensor_copy(ssq_sb[:, 0:512], ssq0[:])
            nc.vector.tensor_copy(ssq_sb[:, 512:1024], ssq1[:])
            nc.sync.dma_start(ssq_rt[:], ssq_sb[:])

        # ---- phase F: output projection --------------------------------
        with tc.tile_pool(name="sq_p", bufs=2) as sq_p:
            ns_l = sq_p.tile([128, NCH], F32, name="ns_l")
            nc.sync.dma_start(ns_l[:], ssq_rt.rearrange("(c p) -> p c", p=128))
            ns_t = sq_p.tile([128, NCH], F32, name="ns_t")
            nc.scalar.activation(
                ns_t[:], ns_l[:], AF.Sqrt, bias=eps_sb[:, 0:1], scale=1.0 / (H * D)
            )
            ns_sb = sq_p.tile([128, NCH], F32, name="ns_sb")
            nc.vector.reciprocal(ns_sb[:], ns_t[:])

            with tc.tile_pool(name="ow_p", bufs=4) as ow_p, tc.tile_pool(
                name="oo_p", bufs=1
            ) as oo_p, tc.tile_pool(name="ps_o", bufs=1, space="PSUM") as ps_o:
                for n in range(4):
                    po = [
                        ps_o.tile([128, 512], F32, name=f"po{m}") for m in range(NCH)
                    ]
                    for k2 in range(KC // 2):
                        ow_t = ow_p.tile([128, 2, 512], BF16, name="ow_t")
                        nc.sync.dma_start(
                            ow_t[:],
                            owT[
                                k2 * 256 : (k2 + 1) * 256, n * 512 : (n + 1) * 512
                            ].rearrange("(ko p) c -> p ko c", p=128),
                        )
                        for kk in range(2):
                            k = 2 * k2 + kk
                            for m in range(NCH):
                                nc.tensor.matmul(
                                    po[m][:],
                                    gate_sb[:, k, m * 128 : (m + 1) * 128],
                                    ow_t[:, kk, :],
                                    start=(k == 0),
                                    stop=(k == KC - 1),
                                )
                    oo_all = oo_p.tile([128, NCH, 512], F32, name="oo_all")
                    for m in range(NCH):
                        nc.vector.tensor_scalar_mul(
                            oo_all[:, m, :], po[m][:], ns_sb[:, m : m + 1]
                        )
                    nc.sync.dma_start(
                        out[:, n * 512 : (n + 1) * 512].rearrange(
                            "(m p) c -> p m c", p=128
                        ),
                        oo_all[:],
                    )
        _eg_ctx.close()
        _g_ctx.close()
        _ce_ctx.close()
        _ys_ctx.close()
        _q_ctx.close()
        _xt_ctx.close()
        _c_ctx.close()
    nc.finalize()
    return nc


_CACHE = {}


def _get_nc():
    if "nc" not in _CACHE:
        _CACHE["nc"] = _build_nc()
    return _CACHE["nc"]


def _host_prep(hidden_states, qkv_w, out_w, gate_w, norm_w):
    slope, qd, kd, dd, bd = _decay()
    w3 = qkv_w.reshape(H, 3 * D, HID)
    wq = w3[:, 0:D, :].reshape(H * D, HID)
    wk = w3[:, D : 2 * D, :].reshape(H * D, HID)
    wv = w3[:, 2 * D : 3 * D, :].reshape(H * D, HID)
    # wqkT[:, g, 0:128] = q dims of heads 2g,2g+1; [:, g, 128:256] = k dims
    wqkT = np.concatenate(
        [
            np.ascontiguousarray(wq.T).reshape(HID, G, 128),
            np.ascontiguousarray(wk.T).reshape(HID, G, 128),
        ],
        axis=2,
    ).astype(BF)
    wvT = np.ascontiguousarray(wv.T).astype(BF)
    gwT = np.ascontiguousarray(gate_w.T).astype(BF)
    owT = np.ascontiguousarray(out_w.T).astype(BF)

    # ddm[g, p, h*512 + jc*256 + i] = dd[2g+h, i, jc*128+p]
    dd_t = dd.transpose(0, 2, 1)  # (H, j, i)
    ddm = np.ascontiguousarray(
        dd_t.reshape(G, 2, 2, 128, BS).transpose(0, 3, 1, 2, 4).reshape(G, 128, 1024)
    ).astype(BF)
    # qdm[p, g, i] = qd[2g + p//64, i]
    qdm = np.ascontiguousarray(
        np.broadcast_to(qd.reshape(G, 2, 1, BS), (G, 2, 64, BS))
        .transpose(1, 2, 0, 3)
        .reshape(128, G, BS)
    ).astype(BF)
    # kdm[p, 2h+parity] = kd[h, parity*128+p]
    kdm = np.ascontiguousarray(
        kd.reshape(H, 2, 128).transpose(2, 0, 1).reshape(128, 2 * H)
    ).astype(np.float32)
    # bdm[p, g] = bd[2g + p//64]
    bdm = np.ascontiguousarray(
        np.broadcast_to(bd.reshape(G, 2, 1), (G, 2, 64)).transpose(1, 2, 0).reshape(128, G)
    ).astype(np.float32)
    nw = np.ascontiguousarray(norm_w.reshape(16, 128).T).astype(np.float32)

    shared = dict(wqkT=wqkT, wvT=wvT, gwT=gwT, owT=owT, ddm=ddm, qdm=qdm,
                  kdm=kdm, bdm=bdm, nw=nw)
    in_maps = []
    for c in range(NC):
        bb, p = c // 4, c % 4
        hsT = np.ascontiguousarray(
            hidden_states[bb, p * T : (p + 1) * T, :].T
        ).astype(BF)
        sw = np.zeros((H, NC), dtype=np.float64)
        for cc in range(NC):
            if cc // 4 == bb and cc % 4 < p:
                sw[:, cc] = bd ** (4.0 * (p - 1 - (cc % 4)))
        # swm[p_, cc*G+g] = sw[2g + p_//64, cc]  (cc-major)
        swm = np.ascontiguousarray(
            np.broadcast_to(sw.reshape(G, 2, 1, NC), (G, 2, 64, NC))
            .transpose(1, 2, 3, 0)
            .reshape(128, NC * G)
        ).astype(np.float32)
        in_maps.append(dict(hsT=hsT, swm=swm, **shared))
    return in_maps


def _run(inputs, trace=False):
    nc = _get_nc()
    in_maps = _host_prep(
        np.asarray(inputs["hidden_states"], dtype=np.float32),
        np.asarray(inputs["qkv_w"], dtype=np.float32),
        np.asarray(inputs["out_w"], dtype=np.float32),
        np.asarray(inputs["gate_w"], dtype=np.float32),
        np.asarray(inputs["norm_w"], dtype=np.float32),
    )
    res = run_bass_kernel_spmd(nc, in_maps, core_ids=list(range(NC)), trace=trace)
    full = np.empty((B, S, HID), dtype=np.float32)
    for c in range(NC):
        bb, p = c // 4, c % 4
        full[bb, p * T : (p + 1) * T, :] = res.results[c]["out"]
    return full, res


def kernel(**inputs):
    return _run(inputs, trace=False)[0]


def kernel_traced(**inputs):
    full, res = _run(inputs, trace=True)
    return full, res.exec_time_ns
